# revision 17
# baseline (speedup 1.0000x reference)
"""ChannelMHSA on Trainium2 (Bass/Tile), data-parallel over batch on 8 cores.

Reference computation (per batch b of x [N, C]):
    qkv  = x @ w_qkv                      # [N, 3C], columns ordered (s, h, d)
    q, k, v per head h: [N, D]
    z_h  = k_h^T @ v_h / sqrt(D)          # [D, D]
    A_h  = softmax(z_h, axis=-1)
    T_h  = A_h @ q_h^T                    # [D, N]
    out[n, h*D+d] = T_h[d, n]
    y    = out @ w_out                    # [N, C]

b_qkv / b_out are all-zero by construction (see input spec) and are ignored.

Kernel layout choices per core (BS=4 batches):
  - Everything runs in bf16 on the PE (1 cycle/row at any free size) with
    fp32 PSUM accumulation; measured end-to-end error vs the fp32
    reference ~8e-3 (tolerance 2e-2). Host pre-casts x / w_qkv / w_out to
    bf16, which also halves the startup DMA bytes.
  - x is transposed on the HOST: the kernel uploads xT [C, N] per batch
    directly, so the PE never runs transpose matmuls and there is no
    xin-DMA stall at batch boundaries. xt_pool holds two batches so batch
    b+1's xT prefetches during batch b's attention/output phases.
  - qT = w_q^T @ x^T computed C-major directly (lhsT = w_q chunks,
    rhs = xT chunks), so q never needs a separate transpose.
  - kv = x @ w_qkv[:, C:3C] computed N-major (lhsT = xT chunks).
  - The output projection is FUSED through the attention:
        y = out @ w_out = q~ @ B,  B_h = A_h^T @ w_out[hD:(h+1)D, :]
    B costs only D-deep contractions (4.6k PE cycles/batch vs 12.3k for
    the A @ q^T route) and y's lhsT becomes qT itself - the T tensor and
    its PSUM->SBUF copies disappear entirely.
  - z per head pair is one chain (lhsT = the pair's k, rhs = the pair's v,
    free=128), emitted LOOKAHEAD=2 pairs ahead of the softmax so the PE
    never waits on ACT. softmax: one exp over the whole [128,128] zps
    (off-diagonal garbage is harmless and ignored), per-block row-sums on
    DVE (free-axis tensor_reduce), and the 1/sum folds into the DVE copy
    that writes the block-diag bf16 a2 = A^T tile feeding the B matmul.
    No max-shift needed: |z/8| is small enough for fp32 exp.
  - Startup DMA is need-ordered across the two HWDGE queues (sync/scalar)
    as FEW, BIG descriptors: pushes recycle a small semaphore pool, so
    many small transfers serialize delivery. wo pushes are deferred past
    the batch-0 qT phase to keep the scalar engine free for qT copies.
  - y stores are one whole-tile DMA per row chunk, alternating queues.
"""

import os
import sys
from contextlib import ExitStack

import numpy as np

for _p in ("/opt/trn_rl_repo", "/opt/pypackages"):
    if _p not in sys.path:
        sys.path.append(_p)

import concourse.bacc as bacc
import concourse.mybir as mybir
import concourse.tile as tile
from concourse import bass_utils

B, N, C = 32, 1024, 768
H, D = 12, 64
P = 128
NCORES = 8
BS = B // NCORES          # batches per core
KC = C // P               # 6 contraction chunks over C
NM = N // P               # 8 chunks over N
F32 = mybir.dt.float32
BF16 = mybir.dt.bfloat16


def _emit(ctx, tc, xt_d, wqkv_d, wo_d, y_d):
    nc = tc.nc

    const = ctx.enter_context(tc.tile_pool(name="const", bufs=1))
    xt_pool = ctx.enter_context(tc.tile_pool(name="xtp", bufs=2 * KC))
    kv_pool = ctx.enter_context(tc.tile_pool(name="kvp", bufs=8))
    qt_pool = ctx.enter_context(tc.tile_pool(name="qtp", bufs=8))
    b_pool = ctx.enter_context(tc.tile_pool(name="bp", bufs=6))
    y_pool = ctx.enter_context(tc.tile_pool(name="yp", bufs=3))
    sm_pool = ctx.enter_context(tc.tile_pool(name="smp", bufs=6))
    psB = ctx.enter_context(tc.tile_pool(name="psB", bufs=5, space="PSUM"))
    psZ = ctx.enter_context(tc.tile_pool(name="psZ", bufs=3, space="PSUM"))

    # Persistent block-diag lhsT tiles for the B matmul, zeroed once. Only
    # the diagonal blocks are rewritten per pair, so off-diag zeros persist.
    zeros = const.tile([P, P], F32, tag="zeros", name="zeros")
    nc.vector.memset(zeros[:], 0.0)
    a2_tiles = []
    for i in range(2):
        a2t = const.tile([P, P], BF16, tag=f"a2_{i}", name=f"a2_{i}")
        nc.vector.tensor_copy(a2t[:], zeros[:])
        a2_tiles.append(a2t)

    def load_xt(b):
        xT = [xt_pool.tile([P, N], BF16, tag="xT", name=f"xT{b}_{p}")
              for p in range(KC)]
        for p in range(KC):
            nc.sync.dma_start(xT[p][:], xt_d[b, p * P:(p + 1) * P, :])
        return xT

    # Startup DMA is bandwidth-bound, so issue transfers in strict
    # need-order split across the two HWDGE queues: xt(b0) on sync || wq on
    # scalar (they gate the qT phase), then wkv split across both queues
    # (gates kv), then wo and the xt(b1) prefetch which are needed later.
    xt0 = load_xt(0)
    wq = []
    for p in range(KC):
        t = const.tile([P, C], BF16, tag=f"wq{p}", name=f"wq{p}")
        nc.scalar.dma_start(t[:], wqkv_d[p * P:(p + 1) * P, 0:C])
        wq.append(t)
    wkv = [const.tile([P, 2 * C], BF16, tag=f"wkv{p}", name=f"wkv{p}")
           for p in range(KC)]
    for p in range(KC):
        eng = nc.sync if p % 2 == 0 else nc.scalar
        eng.dma_start(wkv[p][:], wqkv_d[p * P:(p + 1) * P, C:3 * C])
    # wo tiles are created here but their DMAs are emitted after the batch-0
    # kv phase: descriptor pushes cost ~0.6 us each on the issuing engine,
    # and the scalar engine must not be busy pushing while the qT copies run.
    wo = [const.tile([P, C], BF16, tag=f"wo{p}", name=f"wo{p}")
          for p in range(KC)]

    xt_next = xt0

    for b in range(BS):
        xT = xt_next

        # ---- Phase B1: qT = w_q^T @ x^T, C-major (w_q lands first) ----
        qT = []
        for po in range(KC):
            qtt = qt_pool.tile([P, N], BF16, tag="qT", name=f"qT{b}_{po}")
            qT.append(qtt)
            for nf in range(2):
                ps = psB.tile([P, 512], F32, tag="psB", name=f"psqt{b}_{po}_{nf}",
                              space="PSUM")
                for p in range(KC):
                    nc.tensor.matmul(
                        ps[:],
                        wq[p][:, po * P:(po + 1) * P],
                        xT[p][:, nf * 512:(nf + 1) * 512],
                        start=(p == 0), stop=(p == KC - 1))
                if nf == 0:
                    nc.vector.tensor_copy(qtt[:, nf * 512:(nf + 1) * 512], ps[:])
                else:
                    nc.scalar.copy(qtt[:, nf * 512:(nf + 1) * 512], ps[:])

        # ---- Phase B2: kv = x @ w_qkv[:, C:3C], N-major ----
        kv = []
        for m in range(NM):
            kvt = kv_pool.tile([P, 2 * C], BF16, tag="kv", name=f"kv{b}_{m}")
            kv.append(kvt)
            for f in range(3):
                ps = psB.tile([P, 512], F32, tag="psB", name=f"pskv{b}_{m}_{f}",
                              space="PSUM")
                for p in range(KC):
                    nc.tensor.matmul(
                        ps[:],
                        xT[p][:, m * P:(m + 1) * P],
                        wkv[p][:, f * 512:(f + 1) * 512],
                        start=(p == 0), stop=(p == KC - 1))
                if f == 2:
                    nc.scalar.copy(kvt[:, f * 512:(f + 1) * 512], ps[:])
                else:
                    nc.vector.tensor_copy(kvt[:, f * 512:(f + 1) * 512], ps[:])

        # Deferred wo loads (see above).
        if b == 0:
            for p in range(KC):
                nc.scalar.dma_start(wo[p][:], wo_d[p * P:(p + 1) * P, :])

        # Prefetch next batch's xT now: its pool slots free up as the kv
        # chains above retire, and these loads sit AHEAD of this batch's y
        # stores on the sync queue so they can't be head-of-line blocked.
        if b + 1 < BS:
            xt_next = load_xt(b + 1)

        # ---- Phase C: attention -> B_pr = [A^T W]_pair, software-pipelined
        # two head pairs ahead so the PE's z chains cover the ACT/DVE
        # softmax latency. ----
        Bt = []
        LOOKAHEAD = 2
        zps_pair = {}
        for step in range(KC + LOOKAHEAD):
            if step < KC:
                pr = step
                # z for both heads of the pair in one chain: lhsT = the
                # pair's k (M=128), rhs = the pair's v (free=128). Head 2pr
                # lands on psum rows/cols 0:64, head 2pr+1 on 64:128; the
                # off-diag blocks are cross-head garbage that stays unused.
                zps = psZ.tile([P, P], F32, tag="z", name=f"z{b}_{pr}",
                               space="PSUM")
                zps_pair[pr] = zps
                for m in range(NM):
                    nc.tensor.matmul(
                        zps[:],
                        kv[m][:, 2 * pr * D:(2 * pr + 2) * D],
                        kv[m][:, C + 2 * pr * D:C + (2 * pr + 2) * D],
                        start=(m == 0), stop=(m == NM - 1))
            if step < LOOKAHEAD:
                continue
            pr = step - LOOKAHEAD
            a2 = a2_tiles[pr % 2]
            zps = zps_pair.pop(pr)
            # One exp over the whole tile (garbage off-diag included: values
            # are ~exp(+-16), finite in fp32, and never read afterwards).
            aex = sm_pool.tile([P, P], F32, tag="aex", name=f"aex{b}_{pr}")
            nc.scalar.activation(aex[:], zps[:],
                                 mybir.ActivationFunctionType.Exp,
                                 bias=0.0, scale=0.125)
            ssum = sm_pool.tile([P, 1], F32, tag="ssum", name=f"ss{b}_{pr}")
            for j in range(2):
                rb = j * D
                nc.vector.tensor_reduce(ssum[rb:rb + D, :],
                                        aex[rb:rb + D, rb:rb + D],
                                        mybir.AxisListType.X,
                                        mybir.AluOpType.add)
            rinv = sm_pool.tile([P, 1], F32, tag="rinv", name=f"ri{b}_{pr}")
            nc.vector.reciprocal(rinv[:], ssum[:])
            # a2 = A^T for the pair (block-diag, bf16): the softmax 1/sum is
            # applied by the per-partition scale of this copy.
            for j in range(2):
                rb = j * D
                nc.vector.tensor_scalar_mul(a2[rb:rb + D, rb:rb + D],
                                            aex[rb:rb + D, rb:rb + D],
                                            rinv[rb:rb + D, :])
            # B_pr = a2^T @ w_out rows of this pair: contraction depth is
            # only 128 (the pair's d-rows), free = C split in two.
            bt = b_pool.tile([P, C], BF16, tag="B", name=f"B{b}_{pr}")
            Bt.append(bt)
            for f in range(2):
                ps = psB.tile([P, 384], F32, tag="psB", name=f"psb{b}_{pr}_{f}",
                              space="PSUM")
                nc.tensor.matmul(ps[:], a2[:],
                                 wo[pr][:, f * 384:(f + 1) * 384],
                                 start=True, stop=True)
                if f == 0:
                    nc.scalar.copy(bt[:, f * 384:(f + 1) * 384], ps[:])
                else:
                    nc.vector.tensor_copy(bt[:, f * 384:(f + 1) * 384], ps[:])

        # ---- Phase D: y = q~ @ B (lhsT = qT chunks, rhs = B chunks) ----
        for m in range(NM):
            yt = y_pool.tile([P, C], BF16, tag="y", name=f"y{b}_{m}")
            for f in range(2):
                ps = psB.tile([P, 384], F32, tag="psB", name=f"psy{b}_{m}_{f}",
                              space="PSUM")
                for p in range(KC):
                    nc.tensor.matmul(
                        ps[:],
                        qT[p][:, m * P:(m + 1) * P],
                        Bt[p][:, f * 384:(f + 1) * 384],
                        start=(p == 0), stop=(p == KC - 1))
                if f == 0:
                    nc.vector.tensor_copy(yt[:, f * 384:(f + 1) * 384], ps[:])
                else:
                    nc.scalar.copy(yt[:, f * 384:(f + 1) * 384], ps[:])
            # One whole-tile store per row chunk (descriptor pushes are
            # expensive), alternating between the two HWDGE queues so the
            # final batch's writeback drains at full aggregate bandwidth.
            eng = nc.sync if m % 2 == 0 else nc.scalar
            eng.dma_start(y_d[b, m * P:(m + 1) * P, :], yt[:])


_BUILD_CACHE = {}


def build_program():
    if "nc" in _BUILD_CACHE:
        return _BUILD_CACHE["nc"]
    nc = bacc.Bacc("TRN2", target_bir_lowering=False, debug=False,
                   num_devices=NCORES)
    xt_d = nc.dram_tensor("xt", [BS, C, N], BF16, kind="ExternalInput").ap()
    wqkv_d = nc.dram_tensor("w_qkv", [C, 3 * C], BF16, kind="ExternalInput").ap()
    wo_d = nc.dram_tensor("w_out", [C, C], BF16, kind="ExternalInput").ap()
    y_d = nc.dram_tensor("y", [BS, N, C], BF16, kind="ExternalOutput").ap()
    with tile.TileContext(nc) as tc:
        with ExitStack() as ctx:
            _emit(ctx, tc, xt_d, wqkv_d, wo_d, y_d)
    nc.compile()
    _BUILD_CACHE["nc"] = nc
    return nc


def make_in_maps(x, w_qkv, w_out):
    import ml_dtypes
    bf16 = ml_dtypes.bfloat16
    x = np.asarray(x, dtype=np.float32)
    w_qkv = np.ascontiguousarray(np.asarray(w_qkv, dtype=np.float32)).astype(bf16)
    w_out = np.ascontiguousarray(np.asarray(w_out, dtype=np.float32)).astype(bf16)
    return [
        {"xt": np.ascontiguousarray(
            x[i * BS:(i + 1) * BS].transpose(0, 2, 1)).astype(bf16),
         "w_qkv": w_qkv, "w_out": w_out}
        for i in range(NCORES)
    ]


def kernel(x, w_qkv, b_qkv=None, w_out=None, b_out=None, **_unused):
    nc = build_program()
    in_maps = make_in_maps(x, w_qkv, w_out)
    res = bass_utils.run_bass_kernel_spmd(nc, in_maps,
                                          core_ids=list(range(NCORES)))
    y = np.concatenate([res.results[i]["y"] for i in range(NCORES)], axis=0)
    return np.asarray(y, dtype=np.float32)


# revision 18
# speedup vs baseline: 1.0164x; 1.0164x over previous
"""ChannelMHSA on Trainium2 (Bass/Tile), data-parallel over batch on 8 cores.

Reference computation (per batch b of x [N, C]):
    qkv  = x @ w_qkv                      # [N, 3C], columns ordered (s, h, d)
    q, k, v per head h: [N, D]
    z_h  = k_h^T @ v_h / sqrt(D)          # [D, D]
    A_h  = softmax(z_h, axis=-1)
    T_h  = A_h @ q_h^T                    # [D, N]
    out[n, h*D+d] = T_h[d, n]
    y    = out @ w_out                    # [N, C]

b_qkv / b_out are all-zero by construction (see input spec) and are ignored.

Kernel layout choices per core (BS=4 batches):
  - Everything runs in bf16 on the PE (1 cycle/row at any free size) with
    fp32 PSUM accumulation; measured end-to-end error vs the fp32
    reference ~8e-3 (tolerance 2e-2). Host pre-casts x / w_qkv / w_out to
    bf16, which also halves the startup DMA bytes.
  - x is transposed on the HOST: the kernel uploads xT [C, N] per batch
    directly, so the PE never runs transpose matmuls and there is no
    xin-DMA stall at batch boundaries. xt_pool holds two batches so batch
    b+1's xT prefetches during batch b's attention/output phases.
  - qT = w_q^T @ x^T computed C-major directly (lhsT = w_q chunks,
    rhs = xT chunks), so q never needs a separate transpose.
  - kv = x @ w_qkv[:, C:3C] computed N-major (lhsT = xT chunks).
  - The output projection is FUSED through the attention:
        y = out @ w_out = q~ @ B,  B_h = A_h^T @ w_out[hD:(h+1)D, :]
    B costs only D-deep contractions (4.6k PE cycles/batch vs 12.3k for
    the A @ q^T route) and y's lhsT becomes qT itself - the T tensor and
    its PSUM->SBUF copies disappear entirely.
  - z per head pair is one chain (lhsT = the pair's k, rhs = the pair's v,
    free=128), emitted LOOKAHEAD=2 pairs ahead of the softmax so the PE
    never waits on ACT. softmax: one exp over the whole [128,128] zps
    (off-diagonal garbage is harmless and ignored), per-block row-sums on
    DVE (free-axis tensor_reduce), and the 1/sum folds into the DVE copy
    that writes the block-diag bf16 a2 = A^T tile feeding the B matmul.
    No max-shift needed: |z/8| is small enough for fp32 exp.
  - Startup DMA is need-ordered across the two HWDGE queues (sync/scalar)
    as FEW, BIG descriptors: pushes recycle a small semaphore pool, so
    many small transfers serialize delivery. wo pushes are deferred past
    the batch-0 qT phase to keep the scalar engine free for qT copies.
  - y stores are one whole-tile DMA per row chunk, alternating queues.
"""

import os
import sys
from contextlib import ExitStack

import numpy as np

for _p in ("/opt/trn_rl_repo", "/opt/pypackages"):
    if _p not in sys.path:
        sys.path.append(_p)

import concourse.bacc as bacc
import concourse.mybir as mybir
import concourse.tile as tile
from concourse import bass_utils

B, N, C = 32, 1024, 768
H, D = 12, 64
P = 128
NCORES = 8
BS = B // NCORES          # batches per core
KC = C // P               # 6 contraction chunks over C
NM = N // P               # 8 chunks over N
F32 = mybir.dt.float32
BF16 = mybir.dt.bfloat16


def _emit(ctx, tc, xt_d, wqkv_d, wo_d, y_d):
    nc = tc.nc

    const = ctx.enter_context(tc.tile_pool(name="const", bufs=1))
    xt_pool = ctx.enter_context(tc.tile_pool(name="xtp", bufs=2 * KC))
    kv_pool = ctx.enter_context(tc.tile_pool(name="kvp", bufs=8))
    qt_pool = ctx.enter_context(tc.tile_pool(name="qtp", bufs=12))
    b_pool = ctx.enter_context(tc.tile_pool(name="bp", bufs=6))
    y_pool = ctx.enter_context(tc.tile_pool(name="yp", bufs=3))
    sm_pool = ctx.enter_context(tc.tile_pool(name="smp", bufs=6))
    psB = ctx.enter_context(tc.tile_pool(name="psB", bufs=5, space="PSUM"))
    psZ = ctx.enter_context(tc.tile_pool(name="psZ", bufs=3, space="PSUM"))

    # Persistent block-diag lhsT tiles for the B matmul, zeroed once. Only
    # the diagonal blocks are rewritten per pair, so off-diag zeros persist.
    zeros = const.tile([P, P], F32, tag="zeros", name="zeros")
    nc.vector.memset(zeros[:], 0.0)
    a2_tiles = []
    for i in range(2):
        a2t = const.tile([P, P], BF16, tag=f"a2_{i}", name=f"a2_{i}")
        nc.vector.tensor_copy(a2t[:], zeros[:])
        a2_tiles.append(a2t)

    def load_xt(b):
        xT = [xt_pool.tile([P, N], BF16, tag="xT", name=f"xT{b}_{p}")
              for p in range(KC)]
        for p in range(KC):
            nc.sync.dma_start(xT[p][:], xt_d[b, p * P:(p + 1) * P, :])
        return xT

    # Startup DMA is bandwidth-bound, so issue transfers in strict
    # need-order split across the two HWDGE queues: xt(b0) on sync || wq on
    # scalar (they gate the qT phase), then wkv split across both queues
    # (gates kv), then wo and the xt(b1) prefetch which are needed later.
    xt0 = load_xt(0)
    wq = []
    for p in range(KC):
        t = const.tile([P, C], BF16, tag=f"wq{p}", name=f"wq{p}")
        nc.scalar.dma_start(t[:], wqkv_d[p * P:(p + 1) * P, 0:C])
        wq.append(t)
    wkv = [const.tile([P, 2 * C], BF16, tag=f"wkv{p}", name=f"wkv{p}")
           for p in range(KC)]
    for p in range(KC):
        eng = nc.sync if p % 2 == 0 else nc.scalar
        eng.dma_start(wkv[p][:], wqkv_d[p * P:(p + 1) * P, C:3 * C])
    # wo tiles are created here but their DMAs are emitted after the batch-0
    # kv phase: descriptor pushes cost ~0.6 us each on the issuing engine,
    # and the scalar engine must not be busy pushing while the qT copies run.
    wo = [const.tile([P, C], BF16, tag=f"wo{p}", name=f"wo{p}")
          for p in range(KC)]

    xt_next = xt0

    def emit_qt(b, xT):
        qT = []
        for po in range(KC):
            qtt = qt_pool.tile([P, N], BF16, tag="qT", name=f"qT{b}_{po}")
            qT.append(qtt)
            for nf in range(2):
                ps = psB.tile([P, 512], F32, tag="psB", name=f"psqt{b}_{po}_{nf}",
                              space="PSUM")
                for p in range(KC):
                    nc.tensor.matmul(
                        ps[:],
                        wq[p][:, po * P:(po + 1) * P],
                        xT[p][:, nf * 512:(nf + 1) * 512],
                        start=(p == 0), stop=(p == KC - 1))
                if nf == 0:
                    nc.vector.tensor_copy(qtt[:, nf * 512:(nf + 1) * 512], ps[:])
                else:
                    nc.scalar.copy(qtt[:, nf * 512:(nf + 1) * 512], ps[:])
        return qT

    def emit_kv(b, xT):
        kv = []
        for m in range(NM):
            kvt = kv_pool.tile([P, 2 * C], BF16, tag="kv", name=f"kv{b}_{m}")
            kv.append(kvt)
            for f in range(3):
                ps = psB.tile([P, 512], F32, tag="psB", name=f"pskv{b}_{m}_{f}",
                              space="PSUM")
                for p in range(KC):
                    nc.tensor.matmul(
                        ps[:],
                        xT[p][:, m * P:(m + 1) * P],
                        wkv[p][:, f * 512:(f + 1) * 512],
                        start=(p == 0), stop=(p == KC - 1))
                if f == 2:
                    nc.scalar.copy(kvt[:, f * 512:(f + 1) * 512], ps[:])
                else:
                    nc.vector.tensor_copy(kvt[:, f * 512:(f + 1) * 512], ps[:])
        return kv

    def emit_attention(b, kv):
        Bt = []
        LOOKAHEAD = 2
        zps_pair = {}
        for step in range(KC + LOOKAHEAD):
            if step < KC:
                pr = step
                # z for both heads of the pair in one chain: lhsT = the
                # pair's k (M=128), rhs = the pair's v (free=128). Head 2pr
                # lands on psum rows/cols 0:64, head 2pr+1 on 64:128; the
                # off-diag blocks are cross-head garbage that stays unused.
                zps = psZ.tile([P, P], F32, tag="z", name=f"z{b}_{pr}",
                               space="PSUM")
                zps_pair[pr] = zps
                for m in range(NM):
                    nc.tensor.matmul(
                        zps[:],
                        kv[m][:, 2 * pr * D:(2 * pr + 2) * D],
                        kv[m][:, C + 2 * pr * D:C + (2 * pr + 2) * D],
                        start=(m == 0), stop=(m == NM - 1))
            if step < LOOKAHEAD:
                continue
            pr = step - LOOKAHEAD
            a2 = a2_tiles[pr % 2]
            zps = zps_pair.pop(pr)
            # One exp over the whole tile (garbage off-diag included: values
            # are ~exp(+-16), finite in fp32, and never read afterwards).
            aex = sm_pool.tile([P, P], F32, tag="aex", name=f"aex{b}_{pr}")
            nc.scalar.activation(aex[:], zps[:],
                                 mybir.ActivationFunctionType.Exp,
                                 bias=0.0, scale=0.125)
            ssum = sm_pool.tile([P, 1], F32, tag="ssum", name=f"ss{b}_{pr}")
            for j in range(2):
                rb = j * D
                nc.vector.tensor_reduce(ssum[rb:rb + D, :],
                                        aex[rb:rb + D, rb:rb + D],
                                        mybir.AxisListType.X,
                                        mybir.AluOpType.add)
            rinv = sm_pool.tile([P, 1], F32, tag="rinv", name=f"ri{b}_{pr}")
            nc.vector.reciprocal(rinv[:], ssum[:])
            # a2 = A^T for the pair (block-diag, bf16): the softmax 1/sum is
            # applied by the per-partition scale of this copy.
            for j in range(2):
                rb = j * D
                nc.vector.tensor_scalar_mul(a2[rb:rb + D, rb:rb + D],
                                            aex[rb:rb + D, rb:rb + D],
                                            rinv[rb:rb + D, :])
            # B_pr = a2^T @ w_out rows of this pair: contraction depth is
            # only 128 (the pair's d-rows), free = C split in two.
            bt = b_pool.tile([P, C], BF16, tag="B", name=f"B{b}_{pr}")
            Bt.append(bt)
            for f in range(2):
                ps = psB.tile([P, 384], F32, tag="psB", name=f"psb{b}_{pr}_{f}",
                              space="PSUM")
                nc.tensor.matmul(ps[:], a2[:],
                                 wo[pr][:, f * 384:(f + 1) * 384],
                                 start=True, stop=True)
                if f == 0:
                    nc.scalar.copy(bt[:, f * 384:(f + 1) * 384], ps[:])
                else:
                    nc.vector.tensor_copy(bt[:, f * 384:(f + 1) * 384], ps[:])
        return Bt

    def emit_y(b, qT, Bt):
        for m in range(NM):
            yt = y_pool.tile([P, C], BF16, tag="y", name=f"y{b}_{m}")
            for f in range(2):
                ps = psB.tile([P, 384], F32, tag="psB", name=f"psy{b}_{m}_{f}",
                              space="PSUM")
                for p in range(KC):
                    nc.tensor.matmul(
                        ps[:],
                        qT[p][:, m * P:(m + 1) * P],
                        Bt[p][:, f * 384:(f + 1) * 384],
                        start=(p == 0), stop=(p == KC - 1))
                if f == 0:
                    nc.vector.tensor_copy(yt[:, f * 384:(f + 1) * 384], ps[:])
                else:
                    nc.scalar.copy(yt[:, f * 384:(f + 1) * 384], ps[:])
            # One whole-tile store per row chunk (descriptor pushes are
            # expensive), alternating between the two HWDGE queues so the
            # final batch's writeback drains at full aggregate bandwidth.
            eng = nc.sync if m % 2 == 0 else nc.scalar
            eng.dma_start(y_d[b, m * P:(m + 1) * P, :], yt[:])

    # Batch pipeline, software-pipelined by one batch: y(b-1) is emitted
    # AFTER qT(b), so the independent qT chains fill the PE gaps while the
    # DVE/ACT softmax pipeline of attention(b-1) drains.
    prev = None
    for b in range(BS):
        xT = xt_next
        qT = emit_qt(b, xT)
        if prev is not None:
            emit_y(b - 1, *prev)
        kv = emit_kv(b, xT)
        # Deferred wo loads (see above).
        if b == 0:
            for p in range(KC):
                nc.scalar.dma_start(wo[p][:], wo_d[p * P:(p + 1) * P, :])
        # Prefetch next batch's xT now: its pool slots free up as the kv
        # chains above retire, and these loads sit AHEAD of this batch's y
        # stores on the sync queue so they can't be head-of-line blocked.
        if b + 1 < BS:
            xt_next = load_xt(b + 1)
        Bt = emit_attention(b, kv)
        prev = (qT, Bt)
    emit_y(BS - 1, *prev)


_BUILD_CACHE = {}


def build_program():
    if "nc" in _BUILD_CACHE:
        return _BUILD_CACHE["nc"]
    nc = bacc.Bacc("TRN2", target_bir_lowering=False, debug=False,
                   num_devices=NCORES)
    xt_d = nc.dram_tensor("xt", [BS, C, N], BF16, kind="ExternalInput").ap()
    wqkv_d = nc.dram_tensor("w_qkv", [C, 3 * C], BF16, kind="ExternalInput").ap()
    wo_d = nc.dram_tensor("w_out", [C, C], BF16, kind="ExternalInput").ap()
    y_d = nc.dram_tensor("y", [BS, N, C], BF16, kind="ExternalOutput").ap()
    with tile.TileContext(nc) as tc:
        with ExitStack() as ctx:
            _emit(ctx, tc, xt_d, wqkv_d, wo_d, y_d)
    nc.compile()
    _BUILD_CACHE["nc"] = nc
    return nc


def make_in_maps(x, w_qkv, w_out):
    import ml_dtypes
    bf16 = ml_dtypes.bfloat16
    x = np.asarray(x, dtype=np.float32)
    w_qkv = np.ascontiguousarray(np.asarray(w_qkv, dtype=np.float32)).astype(bf16)
    w_out = np.ascontiguousarray(np.asarray(w_out, dtype=np.float32)).astype(bf16)
    return [
        {"xt": np.ascontiguousarray(
            x[i * BS:(i + 1) * BS].transpose(0, 2, 1)).astype(bf16),
         "w_qkv": w_qkv, "w_out": w_out}
        for i in range(NCORES)
    ]


def kernel(x, w_qkv, b_qkv=None, w_out=None, b_out=None, **_unused):
    nc = build_program()
    in_maps = make_in_maps(x, w_qkv, w_out)
    res = bass_utils.run_bass_kernel_spmd(nc, in_maps,
                                          core_ids=list(range(NCORES)))
    y = np.concatenate([res.results[i]["y"] for i in range(NCORES)], axis=0)
    return np.asarray(y, dtype=np.float32)


# revision 22
# speedup vs baseline: 1.0534x; 1.0363x over previous
"""ChannelMHSA on Trainium2 (Bass/Tile), data-parallel over batch on 8 cores.

Reference computation (per batch b of x [N, C]):
    qkv  = x @ w_qkv                      # [N, 3C], columns ordered (s, h, d)
    q, k, v per head h: [N, D]
    z_h  = k_h^T @ v_h / sqrt(D)          # [D, D]
    A_h  = softmax(z_h, axis=-1)
    T_h  = A_h @ q_h^T                    # [D, N]
    out[n, h*D+d] = T_h[d, n]
    y    = out @ w_out                    # [N, C]

b_qkv / b_out are all-zero by construction (see input spec) and are ignored.

Kernel layout choices per core (BS=4 batches):
  - Everything runs in bf16 on the PE (1 cycle/row at any free size) with
    fp32 PSUM accumulation; measured end-to-end error vs the fp32
    reference ~8e-3 (tolerance 2e-2). Host pre-casts x / w_qkv / w_out to
    bf16, which also halves the startup DMA bytes.
  - x is transposed on the HOST: the kernel uploads xT [C, N] per batch
    directly, so the PE never runs transpose matmuls and there is no
    xin-DMA stall at batch boundaries. xt_pool holds two batches so batch
    b+1's xT prefetches during batch b's attention/output phases.
  - qT = w_q^T @ x^T computed C-major directly (lhsT = w_q chunks,
    rhs = xT chunks), so q never needs a separate transpose.
  - kv = x @ w_qkv[:, C:3C] computed N-major (lhsT = xT chunks).
  - The output projection is FUSED through the attention:
        y = out @ w_out = q~ @ B,  B_h = A_h^T @ w_out[hD:(h+1)D, :]
    B costs only D-deep contractions (4.6k PE cycles/batch vs 12.3k for
    the A @ q^T route) and y's lhsT becomes qT itself - the T tensor and
    its PSUM->SBUF copies disappear entirely.
  - z per head pair is one chain (lhsT = the pair's k, rhs = the pair's v,
    free=128), emitted LOOKAHEAD=2 pairs ahead of the softmax so the PE
    never waits on ACT. softmax: one exp over the whole [128,128] zps
    (off-diagonal garbage is harmless and ignored), per-block row-sums on
    DVE (free-axis tensor_reduce), and the 1/sum folds into the DVE copy
    that writes the block-diag bf16 a2 = A^T tile feeding the B matmul.
    No max-shift needed: |z/8| is small enough for fp32 exp.
  - Startup DMA is need-ordered across the two HWDGE queues (sync/scalar)
    as FEW, BIG descriptors: pushes recycle a small semaphore pool, so
    many small transfers serialize delivery. wo pushes are deferred past
    the batch-0 qT phase to keep the scalar engine free for qT copies.
  - y stores are one whole-tile DMA per row chunk, alternating queues.
"""

import os
import sys
from contextlib import ExitStack

import numpy as np

for _p in ("/opt/trn_rl_repo", "/opt/pypackages"):
    if _p not in sys.path:
        sys.path.append(_p)

import concourse.bacc as bacc
import concourse.mybir as mybir
import concourse.tile as tile
from concourse import bass_utils

B, N, C = 32, 1024, 768
H, D = 12, 64
P = 128
NCORES = 8
BS = B // NCORES          # batches per core
KC = C // P               # 6 contraction chunks over C
NM = N // P               # 8 chunks over N
F32 = mybir.dt.float32
BF16 = mybir.dt.bfloat16


def _emit(ctx, tc, xt_d, wqkv_d, wo_d, y_d):
    nc = tc.nc

    const = ctx.enter_context(tc.tile_pool(name="const", bufs=1))
    xt_pool = ctx.enter_context(tc.tile_pool(name="xtp", bufs=2 * KC))
    kv_pool = ctx.enter_context(tc.tile_pool(name="kvp", bufs=8))
    qt_pool = ctx.enter_context(tc.tile_pool(name="qtp", bufs=12))
    b_pool = ctx.enter_context(tc.tile_pool(name="bp", bufs=6))
    y_pool = ctx.enter_context(tc.tile_pool(name="yp", bufs=3))
    sm_pool = ctx.enter_context(tc.tile_pool(name="smp", bufs=6))
    psB = ctx.enter_context(tc.tile_pool(name="psB", bufs=5, space="PSUM"))
    psZ = ctx.enter_context(tc.tile_pool(name="psZ", bufs=3, space="PSUM"))

    # Persistent block-diag lhsT tiles for the B matmul, zeroed once. Only
    # the diagonal blocks are rewritten per pair, so off-diag zeros persist.
    zeros = const.tile([P, P], F32, tag="zeros", name="zeros")
    nc.vector.memset(zeros[:], 0.0)
    a2_tiles = []
    for i in range(2):
        a2t = const.tile([P, P], BF16, tag=f"a2_{i}", name=f"a2_{i}")
        nc.vector.tensor_copy(a2t[:], zeros[:])
        a2_tiles.append(a2t)

    def load_xt(b):
        xT = [xt_pool.tile([P, N], BF16, tag="xT", name=f"xT{b}_{p}")
              for p in range(KC)]
        for p in range(KC):
            nc.sync.dma_start(xT[p][:], xt_d[b, p * P:(p + 1) * P, :])
        return xT

    # Startup DMA is bandwidth-bound, so issue transfers in strict
    # need-order split across the two HWDGE queues: xt(b0) on sync || wq on
    # scalar (they gate the qT phase), then wkv split across both queues
    # (gates kv), then wo and the xt(b1) prefetch which are needed later.
    xt0 = load_xt(0)
    wq = []
    for p in range(KC):
        t = const.tile([P, C], BF16, tag=f"wq{p}", name=f"wq{p}")
        nc.scalar.dma_start(t[:], wqkv_d[p * P:(p + 1) * P, 0:C])
        wq.append(t)
    wkv = [const.tile([P, 2 * C], BF16, tag=f"wkv{p}", name=f"wkv{p}")
           for p in range(KC)]
    for p in range(KC):
        eng = nc.sync if p % 2 == 0 else nc.scalar
        eng.dma_start(wkv[p][:], wqkv_d[p * P:(p + 1) * P, C:3 * C])
    # wo tiles are created here but their DMAs are emitted after the batch-0
    # kv phase: descriptor pushes cost ~0.6 us each on the issuing engine,
    # and the scalar engine must not be busy pushing while the qT copies run.
    wo = [const.tile([P, C], BF16, tag=f"wo{p}", name=f"wo{p}")
          for p in range(KC)]

    xt_next = xt0

    def make_qt(b, xT):
        """Allocate qT tiles now; return (tiles, list of per-chain thunks)
        so the chains can be woven into another phase's PE gaps."""
        qT = [qt_pool.tile([P, N], BF16, tag="qT", name=f"qT{b}_{po}")
              for po in range(KC)]

        def chain(po, nf):
            def th():
                ps = psB.tile([P, 512], F32, tag="psB",
                              name=f"psqt{b}_{po}_{nf}", space="PSUM")
                for p in range(KC):
                    nc.tensor.matmul(
                        ps[:],
                        wq[p][:, po * P:(po + 1) * P],
                        xT[p][:, nf * 512:(nf + 1) * 512],
                        start=(p == 0), stop=(p == KC - 1))
                if nf == 0:
                    nc.vector.tensor_copy(qT[po][:, nf * 512:(nf + 1) * 512],
                                          ps[:])
                else:
                    nc.scalar.copy(qT[po][:, nf * 512:(nf + 1) * 512], ps[:])
            return th

        return qT, [chain(po, nf) for po in range(KC) for nf in range(2)]

    def emit_kv(b, xT):
        kv = []
        for m in range(NM):
            kvt = kv_pool.tile([P, 2 * C], BF16, tag="kv", name=f"kv{b}_{m}")
            kv.append(kvt)
            for f in range(3):
                ps = psB.tile([P, 512], F32, tag="psB", name=f"pskv{b}_{m}_{f}",
                              space="PSUM")
                for p in range(KC):
                    nc.tensor.matmul(
                        ps[:],
                        xT[p][:, m * P:(m + 1) * P],
                        wkv[p][:, f * 512:(f + 1) * 512],
                        start=(p == 0), stop=(p == KC - 1))
                if f == 2:
                    nc.scalar.copy(kvt[:, f * 512:(f + 1) * 512], ps[:])
                else:
                    nc.vector.tensor_copy(kvt[:, f * 512:(f + 1) * 512], ps[:])
        return kv

    def emit_attention(b, kv, filler=()):
        """filler: thunks of independent PE work (next batch's qT chains)
        woven into the pipeline tail, where no z chains remain to keep the
        PE busy while the ACT/DVE softmax of the last pairs drains."""
        fill_iter = iter(filler)
        Bt = []
        LOOKAHEAD = 2
        zps_pair = {}
        for step in range(KC + LOOKAHEAD):
            if step >= KC:
                for _ in range(3):
                    th = next(fill_iter, None)
                    if th is not None:
                        th()
            if step < KC:
                pr = step
                # z for both heads of the pair in one chain: lhsT = the
                # pair's k (M=128), rhs = the pair's v (free=128). Head 2pr
                # lands on psum rows/cols 0:64, head 2pr+1 on 64:128; the
                # off-diag blocks are cross-head garbage that stays unused.
                zps = psZ.tile([P, P], F32, tag="z", name=f"z{b}_{pr}",
                               space="PSUM")
                zps_pair[pr] = zps
                for m in range(NM):
                    nc.tensor.matmul(
                        zps[:],
                        kv[m][:, 2 * pr * D:(2 * pr + 2) * D],
                        kv[m][:, C + 2 * pr * D:C + (2 * pr + 2) * D],
                        start=(m == 0), stop=(m == NM - 1))
            if step < LOOKAHEAD:
                continue
            pr = step - LOOKAHEAD
            a2 = a2_tiles[pr % 2]
            zps = zps_pair.pop(pr)
            # One exp over the whole tile (garbage off-diag included: values
            # are ~exp(+-16), finite in fp32, and never read afterwards).
            aex = sm_pool.tile([P, P], F32, tag="aex", name=f"aex{b}_{pr}")
            nc.scalar.activation(aex[:], zps[:],
                                 mybir.ActivationFunctionType.Exp,
                                 bias=0.0, scale=0.125)
            ssum = sm_pool.tile([P, 1], F32, tag="ssum", name=f"ss{b}_{pr}")
            for j in range(2):
                rb = j * D
                nc.vector.tensor_reduce(ssum[rb:rb + D, :],
                                        aex[rb:rb + D, rb:rb + D],
                                        mybir.AxisListType.X,
                                        mybir.AluOpType.add)
            rinv = sm_pool.tile([P, 1], F32, tag="rinv", name=f"ri{b}_{pr}")
            nc.vector.reciprocal(rinv[:], ssum[:])
            # a2 = A^T for the pair (block-diag, bf16): the softmax 1/sum is
            # applied by the per-partition scale of this copy.
            for j in range(2):
                rb = j * D
                nc.vector.tensor_scalar_mul(a2[rb:rb + D, rb:rb + D],
                                            aex[rb:rb + D, rb:rb + D],
                                            rinv[rb:rb + D, :])
            # B_pr = a2^T @ w_out rows of this pair: contraction depth is
            # only 128 (the pair's d-rows), free = C split in two.
            bt = b_pool.tile([P, C], BF16, tag="B", name=f"B{b}_{pr}")
            Bt.append(bt)
            for f in range(2):
                ps = psB.tile([P, 384], F32, tag="psB", name=f"psb{b}_{pr}_{f}",
                              space="PSUM")
                nc.tensor.matmul(ps[:], a2[:],
                                 wo[pr][:, f * 384:(f + 1) * 384],
                                 start=True, stop=True)
                if f == 0:
                    nc.scalar.copy(bt[:, f * 384:(f + 1) * 384], ps[:])
                else:
                    nc.vector.tensor_copy(bt[:, f * 384:(f + 1) * 384], ps[:])
        for th in fill_iter:
            th()
        return Bt

    def emit_y(b, qT, Bt):
        for m in range(NM):
            yt = y_pool.tile([P, C], BF16, tag="y", name=f"y{b}_{m}")
            for f in range(2):
                ps = psB.tile([P, 384], F32, tag="psB", name=f"psy{b}_{m}_{f}",
                              space="PSUM")
                for p in range(KC):
                    nc.tensor.matmul(
                        ps[:],
                        qT[p][:, m * P:(m + 1) * P],
                        Bt[p][:, f * 384:(f + 1) * 384],
                        start=(p == 0), stop=(p == KC - 1))
                if f == 0:
                    nc.vector.tensor_copy(yt[:, f * 384:(f + 1) * 384], ps[:])
                else:
                    nc.scalar.copy(yt[:, f * 384:(f + 1) * 384], ps[:])
            # One whole-tile store per row chunk (descriptor pushes are
            # expensive), alternating between the two HWDGE queues so the
            # final batch's writeback drains at full aggregate bandwidth.
            eng = nc.sync if m % 2 == 0 else nc.scalar
            eng.dma_start(y_d[b, m * P:(m + 1) * P, :], yt[:])

    # Batch pipeline, software-pipelined by one batch: the NEXT batch's qT
    # chains are woven into the attention tail of the current batch, where
    # the PE would otherwise idle waiting on the ACT/DVE softmax drain.
    qT, qt_thunks = make_qt(0, xt0)
    for th in qt_thunks:
        th()
    xT = xt0
    for b in range(BS):
        kv = emit_kv(b, xT)
        # Deferred wo loads (see above).
        if b == 0:
            for p in range(KC):
                nc.scalar.dma_start(wo[p][:], wo_d[p * P:(p + 1) * P, :])
        # Prefetch next batch's xT now: its pool slots free up as the kv
        # chains above retire, and these loads sit AHEAD of this batch's y
        # stores on the sync queue so they can't be head-of-line blocked.
        if b + 1 < BS:
            xt_next = load_xt(b + 1)
            qT_next, qt_thunks = make_qt(b + 1, xt_next)
        else:
            qT_next, qt_thunks = None, ()
        Bt = emit_attention(b, kv, filler=qt_thunks)
        emit_y(b, qT, Bt)
        qT, xT = qT_next, xt_next


_BUILD_CACHE = {}


def build_program():
    if "nc" in _BUILD_CACHE:
        return _BUILD_CACHE["nc"]
    nc = bacc.Bacc("TRN2", target_bir_lowering=False, debug=False,
                   num_devices=NCORES)
    xt_d = nc.dram_tensor("xt", [BS, C, N], BF16, kind="ExternalInput").ap()
    wqkv_d = nc.dram_tensor("w_qkv", [C, 3 * C], BF16, kind="ExternalInput").ap()
    wo_d = nc.dram_tensor("w_out", [C, C], BF16, kind="ExternalInput").ap()
    y_d = nc.dram_tensor("y", [BS, N, C], BF16, kind="ExternalOutput").ap()
    with tile.TileContext(nc) as tc:
        with ExitStack() as ctx:
            _emit(ctx, tc, xt_d, wqkv_d, wo_d, y_d)
    nc.compile()
    _BUILD_CACHE["nc"] = nc
    return nc


def make_in_maps(x, w_qkv, w_out):
    import ml_dtypes
    bf16 = ml_dtypes.bfloat16
    x = np.asarray(x, dtype=np.float32)
    w_qkv = np.ascontiguousarray(np.asarray(w_qkv, dtype=np.float32)).astype(bf16)
    w_out = np.ascontiguousarray(np.asarray(w_out, dtype=np.float32)).astype(bf16)
    return [
        {"xt": np.ascontiguousarray(
            x[i * BS:(i + 1) * BS].transpose(0, 2, 1)).astype(bf16),
         "w_qkv": w_qkv, "w_out": w_out}
        for i in range(NCORES)
    ]


def kernel(x, w_qkv, b_qkv=None, w_out=None, b_out=None, **_unused):
    nc = build_program()
    in_maps = make_in_maps(x, w_qkv, w_out)
    res = bass_utils.run_bass_kernel_spmd(nc, in_maps,
                                          core_ids=list(range(NCORES)))
    y = np.concatenate([res.results[i]["y"] for i in range(NCORES)], axis=0)
    return np.asarray(y, dtype=np.float32)


# revision 23
# speedup vs baseline: 1.0553x; 1.0018x over previous
"""ChannelMHSA on Trainium2 (Bass/Tile), data-parallel over batch on 8 cores.

Reference computation (per batch b of x [N, C]):
    qkv  = x @ w_qkv                      # [N, 3C], columns ordered (s, h, d)
    q, k, v per head h: [N, D]
    z_h  = k_h^T @ v_h / sqrt(D)          # [D, D]
    A_h  = softmax(z_h, axis=-1)
    T_h  = A_h @ q_h^T                    # [D, N]
    out[n, h*D+d] = T_h[d, n]
    y    = out @ w_out                    # [N, C]

b_qkv / b_out are all-zero by construction (see input spec) and are ignored.

Kernel layout choices per core (BS=4 batches):
  - Everything runs in bf16 on the PE (1 cycle/row at any free size) with
    fp32 PSUM accumulation; measured end-to-end error vs the fp32
    reference ~8e-3 (tolerance 2e-2). Host pre-casts x / w_qkv / w_out to
    bf16, which also halves the startup DMA bytes.
  - x is transposed on the HOST: the kernel uploads xT [C, N] per batch
    directly, so the PE never runs transpose matmuls and there is no
    xin-DMA stall at batch boundaries. xt_pool holds two batches so batch
    b+1's xT prefetches during batch b's attention/output phases.
  - qT = w_q^T @ x^T computed C-major directly (lhsT = w_q chunks,
    rhs = xT chunks), so q never needs a separate transpose.
  - kv = x @ w_qkv[:, C:3C] computed N-major (lhsT = xT chunks).
  - The output projection is FUSED through the attention:
        y = out @ w_out = q~ @ B,  B_h = A_h^T @ w_out[hD:(h+1)D, :]
    B costs only D-deep contractions (4.6k PE cycles/batch vs 12.3k for
    the A @ q^T route) and y's lhsT becomes qT itself - the T tensor and
    its PSUM->SBUF copies disappear entirely.
  - z per head pair is one chain (lhsT = the pair's k, rhs = the pair's v,
    free=128), emitted LOOKAHEAD=2 pairs ahead of the softmax so the PE
    never waits on ACT. softmax: one exp over the whole [128,128] zps
    (off-diagonal garbage is harmless and ignored), per-block row-sums on
    DVE (free-axis tensor_reduce), and the 1/sum folds into the DVE copy
    that writes the block-diag bf16 a2 = A^T tile feeding the B matmul.
    No max-shift needed: |z/8| is small enough for fp32 exp.
  - Startup DMA is need-ordered across the two HWDGE queues (sync/scalar)
    as FEW, BIG descriptors: pushes recycle a small semaphore pool, so
    many small transfers serialize delivery. wo pushes are deferred past
    the batch-0 qT phase to keep the scalar engine free for qT copies.
  - y stores are one whole-tile DMA per row chunk, alternating queues.
"""

import os
import sys
from contextlib import ExitStack

import numpy as np

for _p in ("/opt/trn_rl_repo", "/opt/pypackages"):
    if _p not in sys.path:
        sys.path.append(_p)

import concourse.bacc as bacc
import concourse.mybir as mybir
import concourse.tile as tile
from concourse import bass_utils

B, N, C = 32, 1024, 768
H, D = 12, 64
P = 128
NCORES = 8
BS = B // NCORES          # batches per core
KC = C // P               # 6 contraction chunks over C
NM = N // P               # 8 chunks over N
F32 = mybir.dt.float32
BF16 = mybir.dt.bfloat16


def _emit(ctx, tc, xt_d, wqkv_d, wo_d, y_d):
    nc = tc.nc

    const = ctx.enter_context(tc.tile_pool(name="const", bufs=1))
    xt_pool = ctx.enter_context(tc.tile_pool(name="xtp", bufs=2 * KC))
    kv_pool = ctx.enter_context(tc.tile_pool(name="kvp", bufs=8))
    qt_pool = ctx.enter_context(tc.tile_pool(name="qtp", bufs=12))
    b_pool = ctx.enter_context(tc.tile_pool(name="bp", bufs=6))
    y_pool = ctx.enter_context(tc.tile_pool(name="yp", bufs=3))
    sm_pool = ctx.enter_context(tc.tile_pool(name="smp", bufs=6))
    psB = ctx.enter_context(tc.tile_pool(name="psB", bufs=3, space="PSUM"))
    # Dedicated PSUM ring for qT chains: they are woven into other phases
    # as filler, and sharing a ring with kv/B/y psums would stall them on
    # those phases' unfinished PSUM->SBUF copies (ring-wrap head-of-line).
    psQ = ctx.enter_context(tc.tile_pool(name="psQ", bufs=2, space="PSUM"))
    psZ = ctx.enter_context(tc.tile_pool(name="psZ", bufs=3, space="PSUM"))

    # Persistent block-diag lhsT tiles for the B matmul, zeroed once. Only
    # the diagonal blocks are rewritten per pair, so off-diag zeros persist.
    zeros = const.tile([P, P], F32, tag="zeros", name="zeros")
    nc.vector.memset(zeros[:], 0.0)
    a2_tiles = []
    for i in range(2):
        a2t = const.tile([P, P], BF16, tag=f"a2_{i}", name=f"a2_{i}")
        nc.vector.tensor_copy(a2t[:], zeros[:])
        a2_tiles.append(a2t)

    def load_xt(b):
        xT = [xt_pool.tile([P, N], BF16, tag="xT", name=f"xT{b}_{p}")
              for p in range(KC)]
        for p in range(KC):
            nc.sync.dma_start(xT[p][:], xt_d[b, p * P:(p + 1) * P, :])
        return xT

    # Startup DMA is bandwidth-bound, so issue transfers in strict
    # need-order split across the two HWDGE queues: xt(b0) on sync || wq on
    # scalar (they gate the qT phase), then wkv split across both queues
    # (gates kv), then wo and the xt(b1) prefetch which are needed later.
    xt0 = load_xt(0)
    wq = []
    for p in range(KC):
        t = const.tile([P, C], BF16, tag=f"wq{p}", name=f"wq{p}")
        nc.scalar.dma_start(t[:], wqkv_d[p * P:(p + 1) * P, 0:C])
        wq.append(t)
    wkv = [const.tile([P, 2 * C], BF16, tag=f"wkv{p}", name=f"wkv{p}")
           for p in range(KC)]
    for p in range(KC):
        eng = nc.sync if p % 2 == 0 else nc.scalar
        eng.dma_start(wkv[p][:], wqkv_d[p * P:(p + 1) * P, C:3 * C])
    # wo tiles are created here but their DMAs are emitted after the batch-0
    # kv phase: descriptor pushes cost ~0.6 us each on the issuing engine,
    # and the scalar engine must not be busy pushing while the qT copies run.
    wo = [const.tile([P, C], BF16, tag=f"wo{p}", name=f"wo{p}")
          for p in range(KC)]

    xt_next = xt0

    def make_qt(b, xT):
        """Allocate qT tiles now; return (tiles, list of per-chain thunks)
        so the chains can be woven into another phase's PE gaps."""
        qT = [qt_pool.tile([P, N], BF16, tag="qT", name=f"qT{b}_{po}")
              for po in range(KC)]

        def chain(po, nf):
            def th():
                ps = psQ.tile([P, 512], F32, tag="psQ",
                              name=f"psqt{b}_{po}_{nf}", space="PSUM")
                for p in range(KC):
                    nc.tensor.matmul(
                        ps[:],
                        wq[p][:, po * P:(po + 1) * P],
                        xT[p][:, nf * 512:(nf + 1) * 512],
                        start=(p == 0), stop=(p == KC - 1))
                if nf == 0:
                    nc.vector.tensor_copy(qT[po][:, nf * 512:(nf + 1) * 512],
                                          ps[:])
                else:
                    nc.scalar.copy(qT[po][:, nf * 512:(nf + 1) * 512], ps[:])
            return th

        return qT, [chain(po, nf) for po in range(KC) for nf in range(2)]

    def emit_kv(b, xT):
        kv = []
        for m in range(NM):
            kvt = kv_pool.tile([P, 2 * C], BF16, tag="kv", name=f"kv{b}_{m}")
            kv.append(kvt)
            for f in range(3):
                ps = psB.tile([P, 512], F32, tag="psB", name=f"pskv{b}_{m}_{f}",
                              space="PSUM")
                for p in range(KC):
                    nc.tensor.matmul(
                        ps[:],
                        xT[p][:, m * P:(m + 1) * P],
                        wkv[p][:, f * 512:(f + 1) * 512],
                        start=(p == 0), stop=(p == KC - 1))
                if f == 2:
                    nc.scalar.copy(kvt[:, f * 512:(f + 1) * 512], ps[:])
                else:
                    nc.vector.tensor_copy(kvt[:, f * 512:(f + 1) * 512], ps[:])
        return kv

    def emit_attention(b, kv, filler=()):
        """filler: thunks of independent PE work (next batch's qT chains)
        woven into the pipeline tail, where no z chains remain to keep the
        PE busy while the ACT/DVE softmax of the last pairs drains."""
        fill_iter = iter(filler)
        Bt = []
        LOOKAHEAD = 2
        zps_pair = {}
        for step in range(KC + LOOKAHEAD):
            if step >= KC - 1:
                for _ in range(4):
                    th = next(fill_iter, None)
                    if th is not None:
                        th()
            if step < KC:
                pr = step
                # z for both heads of the pair in one chain: lhsT = the
                # pair's k (M=128), rhs = the pair's v (free=128). Head 2pr
                # lands on psum rows/cols 0:64, head 2pr+1 on 64:128; the
                # off-diag blocks are cross-head garbage that stays unused.
                zps = psZ.tile([P, P], F32, tag="z", name=f"z{b}_{pr}",
                               space="PSUM")
                zps_pair[pr] = zps
                for m in range(NM):
                    nc.tensor.matmul(
                        zps[:],
                        kv[m][:, 2 * pr * D:(2 * pr + 2) * D],
                        kv[m][:, C + 2 * pr * D:C + (2 * pr + 2) * D],
                        start=(m == 0), stop=(m == NM - 1))
            if step < LOOKAHEAD:
                continue
            pr = step - LOOKAHEAD
            a2 = a2_tiles[pr % 2]
            zps = zps_pair.pop(pr)
            # One exp over the whole tile (garbage off-diag included: values
            # are ~exp(+-16), finite in fp32, and never read afterwards).
            aex = sm_pool.tile([P, P], F32, tag="aex", name=f"aex{b}_{pr}")
            nc.scalar.activation(aex[:], zps[:],
                                 mybir.ActivationFunctionType.Exp,
                                 bias=0.0, scale=0.125)
            ssum = sm_pool.tile([P, 1], F32, tag="ssum", name=f"ss{b}_{pr}")
            for j in range(2):
                rb = j * D
                nc.vector.tensor_reduce(ssum[rb:rb + D, :],
                                        aex[rb:rb + D, rb:rb + D],
                                        mybir.AxisListType.X,
                                        mybir.AluOpType.add)
            rinv = sm_pool.tile([P, 1], F32, tag="rinv", name=f"ri{b}_{pr}")
            nc.vector.reciprocal(rinv[:], ssum[:])
            # a2 = A^T for the pair (block-diag, bf16): the softmax 1/sum is
            # applied by the per-partition scale of this copy.
            for j in range(2):
                rb = j * D
                nc.vector.tensor_scalar_mul(a2[rb:rb + D, rb:rb + D],
                                            aex[rb:rb + D, rb:rb + D],
                                            rinv[rb:rb + D, :])
            # B_pr = a2^T @ w_out rows of this pair: contraction depth is
            # only 128 (the pair's d-rows), free = C split in two.
            bt = b_pool.tile([P, C], BF16, tag="B", name=f"B{b}_{pr}")
            Bt.append(bt)
            for f in range(2):
                ps = psB.tile([P, 384], F32, tag="psB", name=f"psb{b}_{pr}_{f}",
                              space="PSUM")
                nc.tensor.matmul(ps[:], a2[:],
                                 wo[pr][:, f * 384:(f + 1) * 384],
                                 start=True, stop=True)
                if f == 0:
                    nc.scalar.copy(bt[:, f * 384:(f + 1) * 384], ps[:])
                else:
                    nc.vector.tensor_copy(bt[:, f * 384:(f + 1) * 384], ps[:])
        for th in fill_iter:
            th()
        return Bt

    def emit_y(b, qT, Bt):
        for m in range(NM):
            yt = y_pool.tile([P, C], BF16, tag="y", name=f"y{b}_{m}")
            for f in range(2):
                ps = psB.tile([P, 384], F32, tag="psB", name=f"psy{b}_{m}_{f}",
                              space="PSUM")
                for p in range(KC):
                    nc.tensor.matmul(
                        ps[:],
                        qT[p][:, m * P:(m + 1) * P],
                        Bt[p][:, f * 384:(f + 1) * 384],
                        start=(p == 0), stop=(p == KC - 1))
                if f == 0:
                    nc.vector.tensor_copy(yt[:, f * 384:(f + 1) * 384], ps[:])
                else:
                    nc.scalar.copy(yt[:, f * 384:(f + 1) * 384], ps[:])
            # One whole-tile store per row chunk (descriptor pushes are
            # expensive), alternating between the two HWDGE queues so the
            # final batch's writeback drains at full aggregate bandwidth.
            eng = nc.sync if m % 2 == 0 else nc.scalar
            eng.dma_start(y_d[b, m * P:(m + 1) * P, :], yt[:])

    # Batch pipeline, software-pipelined by one batch: the NEXT batch's qT
    # chains are woven into the attention tail of the current batch, where
    # the PE would otherwise idle waiting on the ACT/DVE softmax drain.
    qT, qt_thunks = make_qt(0, xt0)
    for th in qt_thunks:
        th()
    xT = xt0
    for b in range(BS):
        kv = emit_kv(b, xT)
        # Deferred wo loads (see above).
        if b == 0:
            for p in range(KC):
                nc.scalar.dma_start(wo[p][:], wo_d[p * P:(p + 1) * P, :])
        # Prefetch next batch's xT now: its pool slots free up as the kv
        # chains above retire, and these loads sit AHEAD of this batch's y
        # stores on the sync queue so they can't be head-of-line blocked.
        if b + 1 < BS:
            xt_next = load_xt(b + 1)
            qT_next, qt_thunks = make_qt(b + 1, xt_next)
        else:
            qT_next, qt_thunks = None, ()
        Bt = emit_attention(b, kv, filler=qt_thunks)
        emit_y(b, qT, Bt)
        qT, xT = qT_next, xt_next


_BUILD_CACHE = {}


def build_program():
    if "nc" in _BUILD_CACHE:
        return _BUILD_CACHE["nc"]
    nc = bacc.Bacc("TRN2", target_bir_lowering=False, debug=False,
                   num_devices=NCORES)
    xt_d = nc.dram_tensor("xt", [BS, C, N], BF16, kind="ExternalInput").ap()
    wqkv_d = nc.dram_tensor("w_qkv", [C, 3 * C], BF16, kind="ExternalInput").ap()
    wo_d = nc.dram_tensor("w_out", [C, C], BF16, kind="ExternalInput").ap()
    y_d = nc.dram_tensor("y", [BS, N, C], BF16, kind="ExternalOutput").ap()
    with tile.TileContext(nc) as tc:
        with ExitStack() as ctx:
            _emit(ctx, tc, xt_d, wqkv_d, wo_d, y_d)
    nc.compile()
    _BUILD_CACHE["nc"] = nc
    return nc


def make_in_maps(x, w_qkv, w_out):
    import ml_dtypes
    bf16 = ml_dtypes.bfloat16
    x = np.asarray(x, dtype=np.float32)
    w_qkv = np.ascontiguousarray(np.asarray(w_qkv, dtype=np.float32)).astype(bf16)
    w_out = np.ascontiguousarray(np.asarray(w_out, dtype=np.float32)).astype(bf16)
    return [
        {"xt": np.ascontiguousarray(
            x[i * BS:(i + 1) * BS].transpose(0, 2, 1)).astype(bf16),
         "w_qkv": w_qkv, "w_out": w_out}
        for i in range(NCORES)
    ]


def kernel(x, w_qkv, b_qkv=None, w_out=None, b_out=None, **_unused):
    nc = build_program()
    in_maps = make_in_maps(x, w_qkv, w_out)
    res = bass_utils.run_bass_kernel_spmd(nc, in_maps,
                                          core_ids=list(range(NCORES)))
    y = np.concatenate([res.results[i]["y"] for i in range(NCORES)], axis=0)
    return np.asarray(y, dtype=np.float32)


# revision 24
# speedup vs baseline: 1.0671x; 1.0112x over previous
"""ChannelMHSA on Trainium2 (Bass/Tile), data-parallel over batch on 8 cores.

Reference computation (per batch b of x [N, C]):
    qkv  = x @ w_qkv                      # [N, 3C], columns ordered (s, h, d)
    q, k, v per head h: [N, D]
    z_h  = k_h^T @ v_h / sqrt(D)          # [D, D]
    A_h  = softmax(z_h, axis=-1)
    T_h  = A_h @ q_h^T                    # [D, N]
    out[n, h*D+d] = T_h[d, n]
    y    = out @ w_out                    # [N, C]

b_qkv / b_out are all-zero by construction (see input spec) and are ignored.

Kernel layout choices per core (BS=4 batches):
  - Everything runs in bf16 on the PE (1 cycle/row at any free size) with
    fp32 PSUM accumulation; measured end-to-end error vs the fp32
    reference ~8e-3 (tolerance 2e-2). Host pre-casts x / w_qkv / w_out to
    bf16, which also halves the startup DMA bytes.
  - x is transposed on the HOST: the kernel uploads xT [C, N] per batch
    directly, so the PE never runs transpose matmuls and there is no
    xin-DMA stall at batch boundaries. xt_pool holds two batches so batch
    b+1's xT prefetches during batch b's attention/output phases.
  - qT = w_q^T @ x^T computed C-major directly (lhsT = w_q chunks,
    rhs = xT chunks), so q never needs a separate transpose.
  - kv = x @ w_qkv[:, C:3C] computed N-major (lhsT = xT chunks).
  - The output projection is FUSED through the attention:
        y = out @ w_out = q~ @ B,  B_h = A_h^T @ w_out[hD:(h+1)D, :]
    B costs only D-deep contractions (4.6k PE cycles/batch vs 12.3k for
    the A @ q^T route) and y's lhsT becomes qT itself - the T tensor and
    its PSUM->SBUF copies disappear entirely.
  - z per head pair is one chain (lhsT = the pair's k, rhs = the pair's v,
    free=128), emitted LOOKAHEAD=2 pairs ahead of the softmax so the PE
    never waits on ACT. softmax: one exp over the whole [128,128] zps
    (off-diagonal garbage is harmless and ignored), per-block row-sums on
    DVE (free-axis tensor_reduce), and the 1/sum folds into the DVE copy
    that writes the block-diag bf16 a2 = A^T tile feeding the B matmul.
    No max-shift needed: |z/8| is small enough for fp32 exp.
  - Startup DMA is need-ordered across the two HWDGE queues (sync/scalar)
    as FEW, BIG descriptors: pushes recycle a small semaphore pool, so
    many small transfers serialize delivery. wo pushes are deferred past
    the batch-0 qT phase to keep the scalar engine free for qT copies.
  - y stores are one whole-tile DMA per row chunk, alternating queues.
"""

import os
import sys
from contextlib import ExitStack

import numpy as np

for _p in ("/opt/trn_rl_repo", "/opt/pypackages"):
    if _p not in sys.path:
        sys.path.append(_p)

import concourse.bacc as bacc
import concourse.mybir as mybir
import concourse.tile as tile
from concourse import bass_utils

B, N, C = 32, 1024, 768
H, D = 12, 64
P = 128
NCORES = 8
BS = B // NCORES          # batches per core
KC = C // P               # 6 contraction chunks over C
NM = N // P               # 8 chunks over N
F32 = mybir.dt.float32
BF16 = mybir.dt.bfloat16


def _emit(ctx, tc, xt_d, wqkv_d, wo_d, y_d):
    nc = tc.nc

    const = ctx.enter_context(tc.tile_pool(name="const", bufs=1))
    xt_pool = ctx.enter_context(tc.tile_pool(name="xtp", bufs=2 * KC))
    kv_pool = ctx.enter_context(tc.tile_pool(name="kvp", bufs=8))
    qt_pool = ctx.enter_context(tc.tile_pool(name="qtp", bufs=12))
    b_pool = ctx.enter_context(tc.tile_pool(name="bp", bufs=12))
    y_pool = ctx.enter_context(tc.tile_pool(name="yp", bufs=3))
    sm_pool = ctx.enter_context(tc.tile_pool(name="smp", bufs=6))
    psB = ctx.enter_context(tc.tile_pool(name="psB", bufs=3, space="PSUM"))
    # Dedicated PSUM ring for qT chains: they are woven into other phases
    # as filler, and sharing a ring with kv/B/y psums would stall them on
    # those phases' unfinished PSUM->SBUF copies (ring-wrap head-of-line).
    psQ = ctx.enter_context(tc.tile_pool(name="psQ", bufs=2, space="PSUM"))
    psZ = ctx.enter_context(tc.tile_pool(name="psZ", bufs=3, space="PSUM"))

    # Persistent block-diag lhsT tiles for the B matmul, zeroed once. Only
    # the diagonal blocks are rewritten per pair, so off-diag zeros persist.
    zeros = const.tile([P, P], F32, tag="zeros", name="zeros")
    nc.vector.memset(zeros[:], 0.0)
    a2_tiles = []
    for i in range(2):
        a2t = const.tile([P, P], BF16, tag=f"a2_{i}", name=f"a2_{i}")
        nc.vector.tensor_copy(a2t[:], zeros[:])
        a2_tiles.append(a2t)

    def load_xt(b):
        xT = [xt_pool.tile([P, N], BF16, tag="xT", name=f"xT{b}_{p}")
              for p in range(KC)]
        for p in range(KC):
            nc.sync.dma_start(xT[p][:], xt_d[b, p * P:(p + 1) * P, :])
        return xT

    # Startup DMA is bandwidth-bound, so issue transfers in strict
    # need-order split across the two HWDGE queues: xt(b0) on sync || wq on
    # scalar (they gate the qT phase), then wkv split across both queues
    # (gates kv), then wo and the xt(b1) prefetch which are needed later.
    # xt0 / wq tiles interleave across BOTH queues in p-order: the first
    # qT chain consumes all p chunks, so pairwise arrival (xt_p, wq_p)
    # beats loading each tensor on its own queue back-to-back.
    xt0 = [xt_pool.tile([P, N], BF16, tag="xT", name=f"xT0_{p}")
           for p in range(KC)]
    wq = [const.tile([P, C], BF16, tag=f"wq{p}", name=f"wq{p}")
          for p in range(KC)]
    for p in range(KC):
        e_x = nc.sync if p % 2 == 0 else nc.scalar
        e_w = nc.scalar if p % 2 == 0 else nc.sync
        e_x.dma_start(xt0[p][:], xt_d[0, p * P:(p + 1) * P, :])
        e_w.dma_start(wq[p][:], wqkv_d[p * P:(p + 1) * P, 0:C])
    wkv = [const.tile([P, 2 * C], BF16, tag=f"wkv{p}", name=f"wkv{p}")
           for p in range(KC)]
    for p in range(KC):
        eng = nc.sync if p % 2 == 0 else nc.scalar
        eng.dma_start(wkv[p][:], wqkv_d[p * P:(p + 1) * P, C:3 * C])
    # wo tiles are created here but their DMAs are emitted after the batch-0
    # kv phase: descriptor pushes cost ~0.6 us each on the issuing engine,
    # and the scalar engine must not be busy pushing while the qT copies run.
    wo = [const.tile([P, C], BF16, tag=f"wo{p}", name=f"wo{p}")
          for p in range(KC)]

    xt_next = xt0

    def make_qt(b, xT):
        """Allocate qT tiles now; return (tiles, list of per-chain thunks)
        so the chains can be woven into another phase's PE gaps."""
        qT = [qt_pool.tile([P, N], BF16, tag="qT", name=f"qT{b}_{po}")
              for po in range(KC)]

        def chain(po, nf):
            def th():
                ps = psQ.tile([P, 512], F32, tag="psQ",
                              name=f"psqt{b}_{po}_{nf}", space="PSUM")
                for p in range(KC):
                    nc.tensor.matmul(
                        ps[:],
                        wq[p][:, po * P:(po + 1) * P],
                        xT[p][:, nf * 512:(nf + 1) * 512],
                        start=(p == 0), stop=(p == KC - 1))
                if nf == 0:
                    nc.vector.tensor_copy(qT[po][:, nf * 512:(nf + 1) * 512],
                                          ps[:])
                else:
                    nc.scalar.copy(qT[po][:, nf * 512:(nf + 1) * 512], ps[:])
            return th

        return qT, [chain(po, nf) for po in range(KC) for nf in range(2)]

    def emit_kv(b, xT):
        kv = []
        for m in range(NM):
            kvt = kv_pool.tile([P, 2 * C], BF16, tag="kv", name=f"kv{b}_{m}")
            kv.append(kvt)
            for f in range(3):
                ps = psB.tile([P, 512], F32, tag="psB", name=f"pskv{b}_{m}_{f}",
                              space="PSUM")
                for p in range(KC):
                    nc.tensor.matmul(
                        ps[:],
                        xT[p][:, m * P:(m + 1) * P],
                        wkv[p][:, f * 512:(f + 1) * 512],
                        start=(p == 0), stop=(p == KC - 1))
                if f == 2:
                    nc.scalar.copy(kvt[:, f * 512:(f + 1) * 512], ps[:])
                else:
                    nc.vector.tensor_copy(kvt[:, f * 512:(f + 1) * 512], ps[:])
        return kv

    def emit_attention(b, kv, filler=()):
        """filler: thunks of independent PE work (next batch's qT chains)
        woven into the pipeline tail, where no z chains remain to keep the
        PE busy while the ACT/DVE softmax of the last pairs drains."""
        fill_iter = iter(filler)
        Bt = []
        LOOKAHEAD = 2
        zps_pair = {}
        for step in range(KC + LOOKAHEAD):
            if step >= KC - 1:
                for _ in range(4):
                    th = next(fill_iter, None)
                    if th is not None:
                        th()
            if step < KC:
                pr = step
                # z for both heads of the pair in one chain: lhsT = the
                # pair's k (M=128), rhs = the pair's v (free=128). Head 2pr
                # lands on psum rows/cols 0:64, head 2pr+1 on 64:128; the
                # off-diag blocks are cross-head garbage that stays unused.
                zps = psZ.tile([P, P], F32, tag="z", name=f"z{b}_{pr}",
                               space="PSUM")
                zps_pair[pr] = zps
                for m in range(NM):
                    nc.tensor.matmul(
                        zps[:],
                        kv[m][:, 2 * pr * D:(2 * pr + 2) * D],
                        kv[m][:, C + 2 * pr * D:C + (2 * pr + 2) * D],
                        start=(m == 0), stop=(m == NM - 1))
            if step < LOOKAHEAD:
                continue
            pr = step - LOOKAHEAD
            a2 = a2_tiles[pr % 2]
            zps = zps_pair.pop(pr)
            # One exp over the whole tile (garbage off-diag included: values
            # are ~exp(+-16), finite in fp32, and never read afterwards).
            aex = sm_pool.tile([P, P], F32, tag="aex", name=f"aex{b}_{pr}")
            nc.scalar.activation(aex[:], zps[:],
                                 mybir.ActivationFunctionType.Exp,
                                 bias=0.0, scale=0.125)
            ssum = sm_pool.tile([P, 1], F32, tag="ssum", name=f"ss{b}_{pr}")
            for j in range(2):
                rb = j * D
                nc.vector.tensor_reduce(ssum[rb:rb + D, :],
                                        aex[rb:rb + D, rb:rb + D],
                                        mybir.AxisListType.X,
                                        mybir.AluOpType.add)
            rinv = sm_pool.tile([P, 1], F32, tag="rinv", name=f"ri{b}_{pr}")
            nc.vector.reciprocal(rinv[:], ssum[:])
            # a2 = A^T for the pair (block-diag, bf16): the softmax 1/sum is
            # applied by the per-partition scale of this copy.
            for j in range(2):
                rb = j * D
                nc.vector.tensor_scalar_mul(a2[rb:rb + D, rb:rb + D],
                                            aex[rb:rb + D, rb:rb + D],
                                            rinv[rb:rb + D, :])
            # B_pr = a2^T @ w_out rows of this pair: contraction depth is
            # only 128 (the pair's d-rows), free = C split in two.
            bt = b_pool.tile([P, C], BF16, tag="B", name=f"B{b}_{pr}")
            Bt.append(bt)
            for f in range(2):
                ps = psB.tile([P, 384], F32, tag="psB", name=f"psb{b}_{pr}_{f}",
                              space="PSUM")
                nc.tensor.matmul(ps[:], a2[:],
                                 wo[pr][:, f * 384:(f + 1) * 384],
                                 start=True, stop=True)
                if f == 0:
                    nc.scalar.copy(bt[:, f * 384:(f + 1) * 384], ps[:])
                else:
                    nc.vector.tensor_copy(bt[:, f * 384:(f + 1) * 384], ps[:])
        for th in fill_iter:
            th()
        return Bt

    def y_chain(b, qT, Bt, m):
        def th():
            yt = y_pool.tile([P, C], BF16, tag="y", name=f"y{b}_{m}")
            for f in range(2):
                ps = psB.tile([P, 384], F32, tag="psB", name=f"psy{b}_{m}_{f}",
                              space="PSUM")
                for p in range(KC):
                    nc.tensor.matmul(
                        ps[:],
                        qT[p][:, m * P:(m + 1) * P],
                        Bt[p][:, f * 384:(f + 1) * 384],
                        start=(p == 0), stop=(p == KC - 1))
                if f == 0:
                    nc.vector.tensor_copy(yt[:, f * 384:(f + 1) * 384], ps[:])
                else:
                    nc.scalar.copy(yt[:, f * 384:(f + 1) * 384], ps[:])
            # One whole-tile store per row chunk (descriptor pushes are
            # expensive), alternating between the two HWDGE queues so the
            # final batch's writeback drains at full aggregate bandwidth.
            eng = nc.sync if m % 2 == 0 else nc.scalar
            eng.dma_start(y_d[b, m * P:(m + 1) * P, :], yt[:])
        return th

    def emit_y(b, qT, Bt, hold=0):
        ths = [y_chain(b, qT, Bt, m) for m in range(NM)]
        for th in ths[:NM - hold]:
            th()
        return ths[NM - hold:]

    # Batch pipeline, software-pipelined by one batch: the NEXT batch's qT
    # chains are woven into the attention tail of the current batch, where
    # the PE would otherwise idle waiting on the ACT/DVE softmax drain.
    qT, qt_thunks = make_qt(0, xt0)
    for th in qt_thunks:
        th()
    xT = xt0
    held_y = ()
    for b in range(BS):
        kv = emit_kv(b, xT)
        # Deferred wo loads (see above).
        if b == 0:
            for p in range(KC):
                nc.scalar.dma_start(wo[p][:], wo_d[p * P:(p + 1) * P, :])
        # Prefetch next batch's xT now: its pool slots free up as the kv
        # chains above retire, and these loads sit AHEAD of this batch's y
        # stores on the sync queue so they can't be head-of-line blocked.
        if b + 1 < BS:
            xt_next = load_xt(b + 1)
            qT_next, qt_thunks = make_qt(b + 1, xt_next)
        else:
            qT_next, qt_thunks = None, ()
        # The last batch has no next-batch qT chains to weave into its
        # attention tail, so the previous batch's held-back y chains (which
        # only need B(b-1), long since done) serve as filler instead.
        Bt = emit_attention(b, kv, filler=list(qt_thunks) + list(held_y))
        held_y = emit_y(b, qT, Bt, hold=4 if b == BS - 2 else 0)
        qT, xT = qT_next, xt_next


_BUILD_CACHE = {}


def build_program():
    if "nc" in _BUILD_CACHE:
        return _BUILD_CACHE["nc"]
    nc = bacc.Bacc("TRN2", target_bir_lowering=False, debug=False,
                   num_devices=NCORES)
    xt_d = nc.dram_tensor("xt", [BS, C, N], BF16, kind="ExternalInput").ap()
    wqkv_d = nc.dram_tensor("w_qkv", [C, 3 * C], BF16, kind="ExternalInput").ap()
    wo_d = nc.dram_tensor("w_out", [C, C], BF16, kind="ExternalInput").ap()
    y_d = nc.dram_tensor("y", [BS, N, C], BF16, kind="ExternalOutput").ap()
    with tile.TileContext(nc) as tc:
        with ExitStack() as ctx:
            _emit(ctx, tc, xt_d, wqkv_d, wo_d, y_d)
    nc.compile()
    _BUILD_CACHE["nc"] = nc
    return nc


def make_in_maps(x, w_qkv, w_out):
    import ml_dtypes
    bf16 = ml_dtypes.bfloat16
    x = np.asarray(x, dtype=np.float32)
    w_qkv = np.ascontiguousarray(np.asarray(w_qkv, dtype=np.float32)).astype(bf16)
    w_out = np.ascontiguousarray(np.asarray(w_out, dtype=np.float32)).astype(bf16)
    return [
        {"xt": np.ascontiguousarray(
            x[i * BS:(i + 1) * BS].transpose(0, 2, 1)).astype(bf16),
         "w_qkv": w_qkv, "w_out": w_out}
        for i in range(NCORES)
    ]


def kernel(x, w_qkv, b_qkv=None, w_out=None, b_out=None, **_unused):
    nc = build_program()
    in_maps = make_in_maps(x, w_qkv, w_out)
    res = bass_utils.run_bass_kernel_spmd(nc, in_maps,
                                          core_ids=list(range(NCORES)))
    y = np.concatenate([res.results[i]["y"] for i in range(NCORES)], axis=0)
    return np.asarray(y, dtype=np.float32)


# revision 25
# speedup vs baseline: 1.0747x; 1.0071x over previous
"""ChannelMHSA on Trainium2 (Bass/Tile), data-parallel over batch on 8 cores.

Reference computation (per batch b of x [N, C]):
    qkv  = x @ w_qkv                      # [N, 3C], columns ordered (s, h, d)
    q, k, v per head h: [N, D]
    z_h  = k_h^T @ v_h / sqrt(D)          # [D, D]
    A_h  = softmax(z_h, axis=-1)
    T_h  = A_h @ q_h^T                    # [D, N]
    out[n, h*D+d] = T_h[d, n]
    y    = out @ w_out                    # [N, C]

b_qkv / b_out are all-zero by construction (see input spec) and are ignored.

Kernel layout choices per core (BS=4 batches):
  - Everything runs in bf16 on the PE (1 cycle/row at any free size) with
    fp32 PSUM accumulation; measured end-to-end error vs the fp32
    reference ~8e-3 (tolerance 2e-2). Host pre-casts x / w_qkv / w_out to
    bf16, which also halves the startup DMA bytes.
  - x is transposed on the HOST: the kernel uploads xT [C, N] per batch
    directly, so the PE never runs transpose matmuls and there is no
    xin-DMA stall at batch boundaries. xt_pool holds two batches so batch
    b+1's xT prefetches during batch b's attention/output phases.
  - qT = w_q^T @ x^T computed C-major directly (lhsT = w_q chunks,
    rhs = xT chunks), so q never needs a separate transpose.
  - kv = x @ w_qkv[:, C:3C] computed N-major (lhsT = xT chunks).
  - The output projection is FUSED through the attention:
        y = out @ w_out = q~ @ B,  B_h = A_h^T @ w_out[hD:(h+1)D, :]
    B costs only D-deep contractions (4.6k PE cycles/batch vs 12.3k for
    the A @ q^T route) and y's lhsT becomes qT itself - the T tensor and
    its PSUM->SBUF copies disappear entirely.
  - z per head pair is one chain (lhsT = the pair's k, rhs = the pair's v,
    free=128), emitted LOOKAHEAD=2 pairs ahead of the softmax so the PE
    never waits on ACT. softmax: one exp over the whole [128,128] zps
    (off-diagonal garbage is harmless and ignored), per-block row-sums on
    DVE (free-axis tensor_reduce), and the 1/sum folds into the DVE copy
    that writes the block-diag bf16 a2 = A^T tile feeding the B matmul.
    No max-shift needed: |z/8| is small enough for fp32 exp.
  - Startup DMA is need-ordered across the two HWDGE queues (sync/scalar)
    as FEW, BIG descriptors: pushes recycle a small semaphore pool, so
    many small transfers serialize delivery. wo pushes are deferred past
    the batch-0 qT phase to keep the scalar engine free for qT copies.
  - y stores are one whole-tile DMA per row chunk, alternating queues.
"""

import os
import sys
from contextlib import ExitStack

import numpy as np

for _p in ("/opt/trn_rl_repo", "/opt/pypackages"):
    if _p not in sys.path:
        sys.path.append(_p)

import concourse.bacc as bacc
import concourse.mybir as mybir
import concourse.tile as tile
from concourse import bass_utils

B, N, C = 32, 1024, 768
H, D = 12, 64
P = 128
NCORES = 8
BS = B // NCORES          # batches per core
KC = C // P               # 6 contraction chunks over C
NM = N // P               # 8 chunks over N
F32 = mybir.dt.float32
BF16 = mybir.dt.bfloat16


def _emit(ctx, tc, xt_d, wqkv_d, wo_d, y_d):
    nc = tc.nc

    const = ctx.enter_context(tc.tile_pool(name="const", bufs=1))
    xt_pool = ctx.enter_context(tc.tile_pool(name="xtp", bufs=2 * KC))
    kv_pool = ctx.enter_context(tc.tile_pool(name="kvp", bufs=8))
    qt_pool = ctx.enter_context(tc.tile_pool(name="qtp", bufs=12))
    b_pool = ctx.enter_context(tc.tile_pool(name="bp", bufs=12))
    y_pool = ctx.enter_context(tc.tile_pool(name="yp", bufs=3))
    sm_pool = ctx.enter_context(tc.tile_pool(name="smp", bufs=6))
    psB = ctx.enter_context(tc.tile_pool(name="psB", bufs=3, space="PSUM"))
    # Dedicated PSUM ring for qT chains: they are woven into other phases
    # as filler, and sharing a ring with kv/B/y psums would stall them on
    # those phases' unfinished PSUM->SBUF copies (ring-wrap head-of-line).
    psQ = ctx.enter_context(tc.tile_pool(name="psQ", bufs=2, space="PSUM"))
    psZ = ctx.enter_context(tc.tile_pool(name="psZ", bufs=3, space="PSUM"))

    # Persistent block-diag lhsT tiles for the B matmul, zeroed once. Only
    # the diagonal blocks are rewritten per pair, so off-diag zeros persist.
    zeros = const.tile([P, P], F32, tag="zeros", name="zeros")
    nc.vector.memset(zeros[:], 0.0)
    a2_tiles = []
    for i in range(2):
        a2t = const.tile([P, P], BF16, tag=f"a2_{i}", name=f"a2_{i}")
        nc.vector.tensor_copy(a2t[:], zeros[:])
        a2_tiles.append(a2t)

    def load_xt(b):
        xT = [xt_pool.tile([P, N], BF16, tag="xT", name=f"xT{b}_{p}")
              for p in range(KC)]
        for p in range(KC):
            nc.sync.dma_start(xT[p][:], xt_d[b, p * P:(p + 1) * P, :])
        return xT

    # Startup DMA is bandwidth-bound, so issue transfers in strict
    # need-order split across the two HWDGE queues: xt(b0) on sync || wq on
    # scalar (they gate the qT phase), then wkv split across both queues
    # (gates kv), then wo and the xt(b1) prefetch which are needed later.
    # xt0 / wq tiles interleave across BOTH queues in p-order: the first
    # qT chain consumes all p chunks, so pairwise arrival (xt_p, wq_p)
    # beats loading each tensor on its own queue back-to-back.
    xt0 = [xt_pool.tile([P, N], BF16, tag="xT", name=f"xT0_{p}")
           for p in range(KC)]
    wq = [const.tile([P, C], BF16, tag=f"wq{p}", name=f"wq{p}")
          for p in range(KC)]
    for p in range(KC):
        e_x = nc.sync if p % 2 == 0 else nc.scalar
        e_w = nc.scalar if p % 2 == 0 else nc.sync
        e_x.dma_start(xt0[p][:], xt_d[0, p * P:(p + 1) * P, :])
        e_w.dma_start(wq[p][:], wqkv_d[p * P:(p + 1) * P, 0:C])
    wkv = [const.tile([P, 2 * C], BF16, tag=f"wkv{p}", name=f"wkv{p}")
           for p in range(KC)]
    for p in range(KC):
        eng = nc.sync if p % 2 == 0 else nc.scalar
        eng.dma_start(wkv[p][:], wqkv_d[p * P:(p + 1) * P, C:3 * C])
    # wo tiles are created here but their DMAs are emitted after the batch-0
    # kv phase: descriptor pushes cost ~0.6 us each on the issuing engine,
    # and the scalar engine must not be busy pushing while the qT copies run.
    wo = [const.tile([P, C], BF16, tag=f"wo{p}", name=f"wo{p}")
          for p in range(KC)]

    xt_next = xt0

    def make_qt(b, xT):
        """Allocate qT tiles now; return (tiles, list of per-chain thunks)
        so the chains can be woven into another phase's PE gaps."""
        qT = [qt_pool.tile([P, N], BF16, tag="qT", name=f"qT{b}_{po}")
              for po in range(KC)]

        def chain(po, nf):
            def th():
                ps = psQ.tile([P, 512], F32, tag="psQ",
                              name=f"psqt{b}_{po}_{nf}", space="PSUM")
                for p in range(KC):
                    nc.tensor.matmul(
                        ps[:],
                        wq[p][:, po * P:(po + 1) * P],
                        xT[p][:, nf * 512:(nf + 1) * 512],
                        start=(p == 0), stop=(p == KC - 1))
                if nf == 0:
                    nc.vector.tensor_copy(qT[po][:, nf * 512:(nf + 1) * 512],
                                          ps[:])
                else:
                    nc.scalar.copy(qT[po][:, nf * 512:(nf + 1) * 512], ps[:])
            return th

        return qT, [chain(po, nf) for po in range(KC) for nf in range(2)]

    def emit_kv(b, xT):
        kv = []
        for m in range(NM):
            kvt = kv_pool.tile([P, 2 * C], BF16, tag="kv", name=f"kv{b}_{m}")
            kv.append(kvt)
            for f in range(3):
                ps = psB.tile([P, 512], F32, tag="psB", name=f"pskv{b}_{m}_{f}",
                              space="PSUM")
                for p in range(KC):
                    nc.tensor.matmul(
                        ps[:],
                        xT[p][:, m * P:(m + 1) * P],
                        wkv[p][:, f * 512:(f + 1) * 512],
                        start=(p == 0), stop=(p == KC - 1))
                if f == 2:
                    nc.scalar.copy(kvt[:, f * 512:(f + 1) * 512], ps[:])
                else:
                    nc.vector.tensor_copy(kvt[:, f * 512:(f + 1) * 512], ps[:])
        return kv

    def emit_attention(b, kv, filler=()):
        """filler: thunks of independent PE work (next batch's qT chains)
        woven into the pipeline tail, where no z chains remain to keep the
        PE busy while the ACT/DVE softmax of the last pairs drains."""
        fill_iter = iter(filler)
        Bt = []
        LOOKAHEAD = 2
        zps_pair = {}
        for step in range(KC + LOOKAHEAD):
            if step >= KC - 2:
                for _ in range(3 if step < KC else 4):
                    th = next(fill_iter, None)
                    if th is not None:
                        th()
            if step < KC:
                pr = step
                # z for both heads of the pair in one chain: lhsT = the
                # pair's k (M=128), rhs = the pair's v (free=128). Head 2pr
                # lands on psum rows/cols 0:64, head 2pr+1 on 64:128; the
                # off-diag blocks are cross-head garbage that stays unused.
                zps = psZ.tile([P, P], F32, tag="z", name=f"z{b}_{pr}",
                               space="PSUM")
                zps_pair[pr] = zps
                for m in range(NM):
                    nc.tensor.matmul(
                        zps[:],
                        kv[m][:, 2 * pr * D:(2 * pr + 2) * D],
                        kv[m][:, C + 2 * pr * D:C + (2 * pr + 2) * D],
                        start=(m == 0), stop=(m == NM - 1))
            if step < LOOKAHEAD:
                continue
            pr = step - LOOKAHEAD
            a2 = a2_tiles[pr % 2]
            zps = zps_pair.pop(pr)
            # One exp over the whole tile (garbage off-diag included: values
            # are ~exp(+-16), finite in fp32, and never read afterwards).
            aex = sm_pool.tile([P, P], F32, tag="aex", name=f"aex{b}_{pr}")
            nc.scalar.activation(aex[:], zps[:],
                                 mybir.ActivationFunctionType.Exp,
                                 bias=0.0, scale=0.125)
            ssum = sm_pool.tile([P, 1], F32, tag="ssum", name=f"ss{b}_{pr}")
            for j in range(2):
                rb = j * D
                nc.vector.tensor_reduce(ssum[rb:rb + D, :],
                                        aex[rb:rb + D, rb:rb + D],
                                        mybir.AxisListType.X,
                                        mybir.AluOpType.add)
            rinv = sm_pool.tile([P, 1], F32, tag="rinv", name=f"ri{b}_{pr}")
            nc.vector.reciprocal(rinv[:], ssum[:])
            # a2 = A^T for the pair (block-diag, bf16): the softmax 1/sum is
            # applied by the per-partition scale of this copy.
            for j in range(2):
                rb = j * D
                nc.vector.tensor_scalar_mul(a2[rb:rb + D, rb:rb + D],
                                            aex[rb:rb + D, rb:rb + D],
                                            rinv[rb:rb + D, :])
            # B_pr = a2^T @ w_out rows of this pair: contraction depth is
            # only 128 (the pair's d-rows), free = C split in two.
            bt = b_pool.tile([P, C], BF16, tag="B", name=f"B{b}_{pr}")
            Bt.append(bt)
            for f in range(2):
                ps = psB.tile([P, 384], F32, tag="psB", name=f"psb{b}_{pr}_{f}",
                              space="PSUM")
                nc.tensor.matmul(ps[:], a2[:],
                                 wo[pr][:, f * 384:(f + 1) * 384],
                                 start=True, stop=True)
                if f == 0:
                    nc.scalar.copy(bt[:, f * 384:(f + 1) * 384], ps[:])
                else:
                    nc.vector.tensor_copy(bt[:, f * 384:(f + 1) * 384], ps[:])
        for th in fill_iter:
            th()
        return Bt

    def y_chain(b, qT, Bt, m):
        def th():
            yt = y_pool.tile([P, C], BF16, tag="y", name=f"y{b}_{m}")
            for f in range(2):
                ps = psB.tile([P, 384], F32, tag="psB", name=f"psy{b}_{m}_{f}",
                              space="PSUM")
                for p in range(KC):
                    nc.tensor.matmul(
                        ps[:],
                        qT[p][:, m * P:(m + 1) * P],
                        Bt[p][:, f * 384:(f + 1) * 384],
                        start=(p == 0), stop=(p == KC - 1))
                if f == 0:
                    nc.vector.tensor_copy(yt[:, f * 384:(f + 1) * 384], ps[:])
                else:
                    nc.scalar.copy(yt[:, f * 384:(f + 1) * 384], ps[:])
            # One whole-tile store per row chunk (descriptor pushes are
            # expensive), alternating between the two HWDGE queues so the
            # final batch's writeback drains at full aggregate bandwidth.
            # The last batch stores halves as they finish: nothing overlaps
            # the final drain, so starting it earlier shortens the tail.
            eng = nc.sync if m % 2 == 0 else nc.scalar
            if b == BS - 1:
                for f in range(2):
                    eng.dma_start(y_d[b, m * P:(m + 1) * P,
                                      f * 384:(f + 1) * 384],
                                  yt[:, f * 384:(f + 1) * 384])
            else:
                eng.dma_start(y_d[b, m * P:(m + 1) * P, :], yt[:])
        return th

    def emit_y(b, qT, Bt, hold=0):
        ths = [y_chain(b, qT, Bt, m) for m in range(NM)]
        for th in ths[:NM - hold]:
            th()
        return ths[NM - hold:]

    # Batch pipeline, software-pipelined by one batch: the NEXT batch's qT
    # chains are woven into the attention tail of the current batch, where
    # the PE would otherwise idle waiting on the ACT/DVE softmax drain.
    qT, qt_thunks = make_qt(0, xt0)
    for th in qt_thunks:
        th()
    xT = xt0
    held_y = ()
    for b in range(BS):
        kv = emit_kv(b, xT)
        # Deferred wo loads (see above).
        if b == 0:
            for p in range(KC):
                nc.scalar.dma_start(wo[p][:], wo_d[p * P:(p + 1) * P, :])
        # Prefetch next batch's xT now: its pool slots free up as the kv
        # chains above retire, and these loads sit AHEAD of this batch's y
        # stores on the sync queue so they can't be head-of-line blocked.
        if b + 1 < BS:
            xt_next = load_xt(b + 1)
            qT_next, qt_thunks = make_qt(b + 1, xt_next)
        else:
            qT_next, qt_thunks = None, ()
        # The last batch has no next-batch qT chains to weave into its
        # attention tail, so the previous batch's held-back y chains (which
        # only need B(b-1), long since done) serve as filler instead.
        Bt = emit_attention(b, kv, filler=list(qt_thunks) + list(held_y))
        held_y = emit_y(b, qT, Bt, hold=6 if b == BS - 2 else 0)
        qT, xT = qT_next, xt_next


_BUILD_CACHE = {}


def build_program():
    if "nc" in _BUILD_CACHE:
        return _BUILD_CACHE["nc"]
    nc = bacc.Bacc("TRN2", target_bir_lowering=False, debug=False,
                   num_devices=NCORES)
    xt_d = nc.dram_tensor("xt", [BS, C, N], BF16, kind="ExternalInput").ap()
    wqkv_d = nc.dram_tensor("w_qkv", [C, 3 * C], BF16, kind="ExternalInput").ap()
    wo_d = nc.dram_tensor("w_out", [C, C], BF16, kind="ExternalInput").ap()
    y_d = nc.dram_tensor("y", [BS, N, C], BF16, kind="ExternalOutput").ap()
    with tile.TileContext(nc) as tc:
        with ExitStack() as ctx:
            _emit(ctx, tc, xt_d, wqkv_d, wo_d, y_d)
    nc.compile()
    _BUILD_CACHE["nc"] = nc
    return nc


def make_in_maps(x, w_qkv, w_out):
    import ml_dtypes
    bf16 = ml_dtypes.bfloat16
    x = np.asarray(x, dtype=np.float32)
    w_qkv = np.ascontiguousarray(np.asarray(w_qkv, dtype=np.float32)).astype(bf16)
    w_out = np.ascontiguousarray(np.asarray(w_out, dtype=np.float32)).astype(bf16)
    return [
        {"xt": np.ascontiguousarray(
            x[i * BS:(i + 1) * BS].transpose(0, 2, 1)).astype(bf16),
         "w_qkv": w_qkv, "w_out": w_out}
        for i in range(NCORES)
    ]


def kernel(x, w_qkv, b_qkv=None, w_out=None, b_out=None, **_unused):
    nc = build_program()
    in_maps = make_in_maps(x, w_qkv, w_out)
    res = bass_utils.run_bass_kernel_spmd(nc, in_maps,
                                          core_ids=list(range(NCORES)))
    y = np.concatenate([res.results[i]["y"] for i in range(NCORES)], axis=0)
    return np.asarray(y, dtype=np.float32)


# revision 26
# speedup vs baseline: 1.1099x; 1.0328x over previous
"""ChannelMHSA on Trainium2 (Bass/Tile), data-parallel over batch on 8 cores.

Reference computation (per batch b of x [N, C]):
    qkv  = x @ w_qkv                      # [N, 3C], columns ordered (s, h, d)
    q, k, v per head h: [N, D]
    z_h  = k_h^T @ v_h / sqrt(D)          # [D, D]
    A_h  = softmax(z_h, axis=-1)
    T_h  = A_h @ q_h^T                    # [D, N]
    out[n, h*D+d] = T_h[d, n]
    y    = out @ w_out                    # [N, C]

b_qkv / b_out are all-zero by construction (see input spec) and are ignored.

Kernel layout choices per core (BS=4 batches):
  - Everything runs in bf16 on the PE (1 cycle/row at any free size) with
    fp32 PSUM accumulation; measured end-to-end error vs the fp32
    reference ~8e-3 (tolerance 2e-2). Host pre-casts x / w_qkv / w_out to
    bf16, which also halves the startup DMA bytes.
  - x is transposed on the HOST: the kernel uploads xT [C, N] per batch
    directly, so the PE never runs transpose matmuls and there is no
    xin-DMA stall at batch boundaries. xt_pool holds two batches so batch
    b+1's xT prefetches during batch b's attention/output phases.
  - qT = w_q^T @ x^T computed C-major directly (lhsT = w_q chunks,
    rhs = xT chunks), so q never needs a separate transpose.
  - kv = x @ w_qkv[:, C:3C] computed N-major (lhsT = xT chunks).
  - The output projection is FUSED through the attention:
        y = out @ w_out = q~ @ B,  B_h = A_h^T @ w_out[hD:(h+1)D, :]
    B costs only D-deep contractions (4.6k PE cycles/batch vs 12.3k for
    the A @ q^T route) and y's lhsT becomes qT itself - the T tensor and
    its PSUM->SBUF copies disappear entirely.
  - z per head pair is one chain (lhsT = the pair's k, rhs = the pair's v,
    free=128), emitted LOOKAHEAD=2 pairs ahead of the softmax so the PE
    never waits on ACT. softmax: one exp over the whole [128,128] zps
    (off-diagonal garbage is harmless and ignored), per-block row-sums on
    DVE (free-axis tensor_reduce), and the 1/sum folds into the DVE copy
    that writes the block-diag bf16 a2 = A^T tile feeding the B matmul.
    No max-shift needed: |z/8| is small enough for fp32 exp.
  - Startup DMA is need-ordered across the two HWDGE queues (sync/scalar)
    as FEW, BIG descriptors: pushes recycle a small semaphore pool, so
    many small transfers serialize delivery. wo pushes are deferred past
    the batch-0 qT phase to keep the scalar engine free for qT copies.
  - y stores are one whole-tile DMA per row chunk, alternating queues.
"""

import os
import sys
from contextlib import ExitStack

import numpy as np

for _p in ("/opt/trn_rl_repo", "/opt/pypackages"):
    if _p not in sys.path:
        sys.path.append(_p)

import concourse.bacc as bacc
import concourse.mybir as mybir
import concourse.tile as tile
from concourse import bass_utils

B, N, C = 32, 1024, 768
H, D = 12, 64
P = 128
NCORES = 8
BS = B // NCORES          # batches per core
KC = C // P               # 6 contraction chunks over C
NM = N // P               # 8 chunks over N
F32 = mybir.dt.float32
BF16 = mybir.dt.bfloat16


def _emit(ctx, tc, xt_d, wqkv_d, wqt_d, wo_d, y_d):
    nc = tc.nc

    const = ctx.enter_context(tc.tile_pool(name="const", bufs=1))
    xt_pool = ctx.enter_context(tc.tile_pool(name="xtp", bufs=2 * KC))
    kv_pool = ctx.enter_context(tc.tile_pool(name="kvp", bufs=16))
    b_pool = ctx.enter_context(tc.tile_pool(name="bp", bufs=24))
    y_pool = ctx.enter_context(tc.tile_pool(name="yp", bufs=3))
    sm_pool = ctx.enter_context(tc.tile_pool(name="smp", bufs=6))
    psB = ctx.enter_context(tc.tile_pool(name="psB", bufs=3, space="PSUM"))
    # Dedicated PSUM ring for qT chains: they are woven into other phases
    # as filler, and sharing a ring with kv/B/y psums would stall them on
    # those phases' unfinished PSUM->SBUF copies (ring-wrap head-of-line).
    psQ = ctx.enter_context(tc.tile_pool(name="psQ", bufs=2, space="PSUM"))
    psZ = ctx.enter_context(tc.tile_pool(name="psZ", bufs=3, space="PSUM"))

    # Persistent block-diag lhsT tiles for the B matmul, zeroed once. Only
    # the diagonal blocks are rewritten per pair, so off-diag zeros persist.
    zeros = const.tile([P, P], F32, tag="zeros", name="zeros")
    nc.vector.memset(zeros[:], 0.0)
    a2_tiles = []
    for i in range(2):
        a2t = const.tile([P, P], BF16, tag=f"a2_{i}", name=f"a2_{i}")
        nc.vector.tensor_copy(a2t[:], zeros[:])
        a2_tiles.append(a2t)

    def load_xt(b):
        xT = [xt_pool.tile([P, N], BF16, tag="xT", name=f"xT{b}_{p}")
              for p in range(KC)]
        for p in range(KC):
            nc.sync.dma_start(xT[p][:], xt_d[b, p * P:(p + 1) * P, :])
        return xT

    # Startup DMA is bandwidth-bound, so issue transfers in strict
    # need-order split across the two HWDGE queues: xt(b0) on sync || wq on
    # scalar (they gate the qT phase), then wkv split across both queues
    # (gates kv), then wo and the xt(b1) prefetch which are needed later.
    # xt0 / wkv tiles interleave across BOTH queues in p-order: the first
    # kv chain consumes all p chunks of both, so pairwise arrival beats
    # loading each tensor on its own queue back-to-back. wqT (W_q
    # transposed, for the G = W_q @ B matmul) and wo are needed only from
    # the attention phase on, so they load after.
    xt0 = [xt_pool.tile([P, N], BF16, tag="xT", name=f"xT0_{p}")
           for p in range(KC)]
    wkv = [const.tile([P, 2 * C], BF16, tag=f"wkv{p}", name=f"wkv{p}")
           for p in range(KC)]
    for p in range(KC):
        e_x = nc.sync if p % 2 == 0 else nc.scalar
        e_w = nc.scalar if p % 2 == 0 else nc.sync
        e_x.dma_start(xt0[p][:], xt_d[0, p * P:(p + 1) * P, :])
        e_w.dma_start(wkv[p][:], wqkv_d[p * P:(p + 1) * P, C:3 * C])
    wo = [const.tile([P, C], BF16, tag=f"wo{p}", name=f"wo{p}")
          for p in range(KC)]
    wqT = [const.tile([P, C], BF16, tag=f"wqT{p}", name=f"wqT{p}")
           for p in range(KC)]
    for p in range(KC):
        eng = nc.sync if p % 2 == 0 else nc.scalar
        eng.dma_start(wo[p][:], wo_d[p * P:(p + 1) * P, :])
        eng2 = nc.scalar if p % 2 == 0 else nc.sync
        eng2.dma_start(wqT[p][:], wqt_d[p * P:(p + 1) * P, :])

    xt_next = xt0

    def make_kv(b, xT):
        """Allocate kv tiles now; return (tiles, list of per-chain thunks)
        so the chains can be woven into another phase's PE gaps."""
        kv = [kv_pool.tile([P, 2 * C], BF16, tag="kv", name=f"kv{b}_{m}")
              for m in range(NM)]

        def chain(m, f):
            def th():
                ps = psQ.tile([P, 512], F32, tag="psQ",
                              name=f"pskv{b}_{m}_{f}", space="PSUM")
                for p in range(KC):
                    nc.tensor.matmul(
                        ps[:],
                        xT[p][:, m * P:(m + 1) * P],
                        wkv[p][:, f * 512:(f + 1) * 512],
                        start=(p == 0), stop=(p == KC - 1))
                if f == 2:
                    nc.scalar.copy(kv[m][:, f * 512:(f + 1) * 512], ps[:])
                else:
                    nc.vector.tensor_copy(kv[m][:, f * 512:(f + 1) * 512],
                                          ps[:])
            return th

        return kv, [chain(m, f) for m in range(NM) for f in range(3)]

    def emit_attention(b, kv, filler=()):
        """filler: thunks of independent PE work (next batch's qT chains)
        woven into the pipeline tail, where no z chains remain to keep the
        PE busy while the ACT/DVE softmax of the last pairs drains."""
        fill_iter = iter(filler)
        Bt = []
        LOOKAHEAD = 2
        zps_pair = {}
        for step in range(KC + LOOKAHEAD):
            if step >= KC - 2:
                for _ in range(3 if step < KC else 4):
                    th = next(fill_iter, None)
                    if th is not None:
                        th()
            if step < KC:
                pr = step
                # z for both heads of the pair in one chain: lhsT = the
                # pair's k (M=128), rhs = the pair's v (free=128). Head 2pr
                # lands on psum rows/cols 0:64, head 2pr+1 on 64:128; the
                # off-diag blocks are cross-head garbage that stays unused.
                zps = psZ.tile([P, P], F32, tag="z", name=f"z{b}_{pr}",
                               space="PSUM")
                zps_pair[pr] = zps
                for m in range(NM):
                    nc.tensor.matmul(
                        zps[:],
                        kv[m][:, 2 * pr * D:(2 * pr + 2) * D],
                        kv[m][:, C + 2 * pr * D:C + (2 * pr + 2) * D],
                        start=(m == 0), stop=(m == NM - 1))
            if step < LOOKAHEAD:
                continue
            pr = step - LOOKAHEAD
            a2 = a2_tiles[pr % 2]
            zps = zps_pair.pop(pr)
            # One exp over the whole tile (garbage off-diag included: values
            # are ~exp(+-16), finite in fp32, and never read afterwards).
            aex = sm_pool.tile([P, P], F32, tag="aex", name=f"aex{b}_{pr}")
            nc.scalar.activation(aex[:], zps[:],
                                 mybir.ActivationFunctionType.Exp,
                                 bias=0.0, scale=0.125)
            ssum = sm_pool.tile([P, 1], F32, tag="ssum", name=f"ss{b}_{pr}")
            for j in range(2):
                rb = j * D
                nc.vector.tensor_reduce(ssum[rb:rb + D, :],
                                        aex[rb:rb + D, rb:rb + D],
                                        mybir.AxisListType.X,
                                        mybir.AluOpType.add)
            rinv = sm_pool.tile([P, 1], F32, tag="rinv", name=f"ri{b}_{pr}")
            nc.vector.reciprocal(rinv[:], ssum[:])
            # a2 = A^T for the pair (block-diag, bf16): the softmax 1/sum is
            # applied by the per-partition scale of this copy.
            for j in range(2):
                rb = j * D
                nc.vector.tensor_scalar_mul(a2[rb:rb + D, rb:rb + D],
                                            aex[rb:rb + D, rb:rb + D],
                                            rinv[rb:rb + D, :])
            # B_pr = a2^T @ w_out rows of this pair: contraction depth is
            # only 128 (the pair's d-rows), free = C split in two.
            bt = b_pool.tile([P, C], BF16, tag="B", name=f"B{b}_{pr}")
            Bt.append(bt)
            for f in range(2):
                ps = psB.tile([P, 384], F32, tag="psB", name=f"psb{b}_{pr}_{f}",
                              space="PSUM")
                nc.tensor.matmul(ps[:], a2[:],
                                 wo[pr][:, f * 384:(f + 1) * 384],
                                 start=True, stop=True)
                if f == 0:
                    nc.scalar.copy(bt[:, f * 384:(f + 1) * 384], ps[:])
                else:
                    nc.vector.tensor_copy(bt[:, f * 384:(f + 1) * 384], ps[:])
        for th in fill_iter:
            th()
        return Bt

    def emit_g(b, Bt):
        G = []
        for po in range(KC):
            gt = b_pool.tile([P, C], BF16, tag="G", name=f"G{b}_{po}")
            G.append(gt)
            for f in range(2):
                ps = psB.tile([P, 384], F32, tag="psB", name=f"psg{b}_{po}_{f}",
                              space="PSUM")
                for p in range(KC):
                    nc.tensor.matmul(
                        ps[:],
                        wqT[p][:, po * P:(po + 1) * P],
                        Bt[p][:, f * 384:(f + 1) * 384],
                        start=(p == 0), stop=(p == KC - 1))
                if f == 0:
                    nc.vector.tensor_copy(gt[:, f * 384:(f + 1) * 384], ps[:])
                else:
                    nc.scalar.copy(gt[:, f * 384:(f + 1) * 384], ps[:])
        return G

    def y_chain(b, xT, G, m):
        def th():
            yt = y_pool.tile([P, C], BF16, tag="y", name=f"y{b}_{m}")
            for f in range(2):
                ps = psB.tile([P, 384], F32, tag="psB", name=f"psy{b}_{m}_{f}",
                              space="PSUM")
                for p in range(KC):
                    nc.tensor.matmul(
                        ps[:],
                        xT[p][:, m * P:(m + 1) * P],
                        G[p][:, f * 384:(f + 1) * 384],
                        start=(p == 0), stop=(p == KC - 1))
                if f == 0:
                    nc.vector.tensor_copy(yt[:, f * 384:(f + 1) * 384], ps[:])
                else:
                    nc.scalar.copy(yt[:, f * 384:(f + 1) * 384], ps[:])
            # One whole-tile store per row chunk (descriptor pushes are
            # expensive), alternating between the two HWDGE queues so the
            # final batch's writeback drains at full aggregate bandwidth.
            # The last batch stores halves as they finish: nothing overlaps
            # the final drain, so starting it earlier shortens the tail.
            eng = nc.sync if m % 2 == 0 else nc.scalar
            if b == BS - 1:
                for f in range(2):
                    eng.dma_start(y_d[b, m * P:(m + 1) * P,
                                      f * 384:(f + 1) * 384],
                                  yt[:, f * 384:(f + 1) * 384])
            else:
                eng.dma_start(y_d[b, m * P:(m + 1) * P, :], yt[:])
        return th

    def emit_y(b, xT, G, hold=0):
        ths = [y_chain(b, xT, G, m) for m in range(NM)]
        for th in ths[:NM - hold]:
            th()
        return ths[NM - hold:]

    # Batch pipeline, software-pipelined by one batch: the NEXT batch's kv
    # chains are woven into the attention tail of the current batch, where
    # the PE would otherwise idle waiting on the ACT/DVE softmax drain; the
    # rest are emitted just before the next attention phase. The last batch
    # weaves the previous batch's held-back y chains instead.
    kv, kv_thunks = make_kv(0, xt0)
    for th in kv_thunks:
        th()
    xT = xt0
    held_y = ()
    for b in range(BS):
        # Prefetch next batch's xT now: its pool slots free up as the kv
        # chains above retire, and these loads sit AHEAD of this batch's y
        # stores on the sync queue so they can't be head-of-line blocked.
        if b + 1 < BS:
            xt_next = load_xt(b + 1)
            kv_next, kv_thunks = make_kv(b + 1, xt_next)
        else:
            kv_next, kv_thunks = None, ()
        n_weave = min(6, len(kv_thunks))
        Bt = emit_attention(b, kv,
                            filler=list(kv_thunks[:n_weave]) + list(held_y))
        G = emit_g(b, Bt)
        held_y = emit_y(b, xT, G, hold=6 if b == BS - 2 else 0)
        for th in kv_thunks[n_weave:]:
            th()
        kv, xT = kv_next, xt_next


_BUILD_CACHE = {}


def build_program():
    if "nc" in _BUILD_CACHE:
        return _BUILD_CACHE["nc"]
    nc = bacc.Bacc("TRN2", target_bir_lowering=False, debug=False,
                   num_devices=NCORES)
    xt_d = nc.dram_tensor("xt", [BS, C, N], BF16, kind="ExternalInput").ap()
    wqkv_d = nc.dram_tensor("w_qkv", [C, 3 * C], BF16, kind="ExternalInput").ap()
    wqt_d = nc.dram_tensor("w_qt", [C, C], BF16, kind="ExternalInput").ap()
    wo_d = nc.dram_tensor("w_out", [C, C], BF16, kind="ExternalInput").ap()
    y_d = nc.dram_tensor("y", [BS, N, C], BF16, kind="ExternalOutput").ap()
    with tile.TileContext(nc) as tc:
        with ExitStack() as ctx:
            _emit(ctx, tc, xt_d, wqkv_d, wqt_d, wo_d, y_d)
    nc.compile()
    _BUILD_CACHE["nc"] = nc
    return nc


def make_in_maps(x, w_qkv, w_out):
    import ml_dtypes
    bf16 = ml_dtypes.bfloat16
    x = np.asarray(x, dtype=np.float32)
    w_qkv = np.asarray(w_qkv, dtype=np.float32)
    w_qt = np.ascontiguousarray(w_qkv[:, :C].T).astype(bf16)
    w_qkv = np.ascontiguousarray(w_qkv).astype(bf16)
    w_out = np.ascontiguousarray(np.asarray(w_out, dtype=np.float32)).astype(bf16)
    return [
        {"xt": np.ascontiguousarray(
            x[i * BS:(i + 1) * BS].transpose(0, 2, 1)).astype(bf16),
         "w_qkv": w_qkv, "w_qt": w_qt, "w_out": w_out}
        for i in range(NCORES)
    ]


def kernel(x, w_qkv, b_qkv=None, w_out=None, b_out=None, **_unused):
    nc = build_program()
    in_maps = make_in_maps(x, w_qkv, w_out)
    res = bass_utils.run_bass_kernel_spmd(nc, in_maps,
                                          core_ids=list(range(NCORES)))
    y = np.concatenate([res.results[i]["y"] for i in range(NCORES)], axis=0)
    return np.asarray(y, dtype=np.float32)


# revision 27
# speedup vs baseline: 1.1330x; 1.0208x over previous
"""ChannelMHSA on Trainium2 (Bass/Tile), data-parallel over batch on 8 cores.

Reference computation (per batch b of x [N, C]):
    qkv  = x @ w_qkv                      # [N, 3C], columns ordered (s, h, d)
    q, k, v per head h: [N, D]
    z_h  = k_h^T @ v_h / sqrt(D)          # [D, D]
    A_h  = softmax(z_h, axis=-1)
    T_h  = A_h @ q_h^T                    # [D, N]
    out[n, h*D+d] = T_h[d, n]
    y    = out @ w_out                    # [N, C]

b_qkv / b_out are all-zero by construction (see input spec) and are ignored.

Kernel layout choices per core (BS=4 batches):
  - Everything runs in bf16 on the PE (1 cycle/row at any free size) with
    fp32 PSUM accumulation; measured end-to-end error vs the fp32
    reference ~8e-3 (tolerance 2e-2). Host pre-casts x / w_qkv / w_out to
    bf16, which also halves the startup DMA bytes.
  - x is transposed on the HOST: the kernel uploads xT [C, N] per batch
    directly, so the PE never runs transpose matmuls and there is no
    xin-DMA stall at batch boundaries. xt_pool holds two batches so batch
    b+1's xT prefetches during batch b's attention/output phases.
  - qT = w_q^T @ x^T computed C-major directly (lhsT = w_q chunks,
    rhs = xT chunks), so q never needs a separate transpose.
  - kv = x @ w_qkv[:, C:3C] computed N-major (lhsT = xT chunks).
  - The output projection is FUSED through the attention:
        y = out @ w_out = q~ @ B,  B_h = A_h^T @ w_out[hD:(h+1)D, :]
    B costs only D-deep contractions (4.6k PE cycles/batch vs 12.3k for
    the A @ q^T route) and y's lhsT becomes qT itself - the T tensor and
    its PSUM->SBUF copies disappear entirely.
  - z per head pair is one chain (lhsT = the pair's k, rhs = the pair's v,
    free=128), emitted LOOKAHEAD=2 pairs ahead of the softmax so the PE
    never waits on ACT. softmax: one exp over the whole [128,128] zps
    (off-diagonal garbage is harmless and ignored), per-block row-sums on
    DVE (free-axis tensor_reduce), and the 1/sum folds into the DVE copy
    that writes the block-diag bf16 a2 = A^T tile feeding the B matmul.
    No max-shift needed: |z/8| is small enough for fp32 exp.
  - Startup DMA is need-ordered across the two HWDGE queues (sync/scalar)
    as FEW, BIG descriptors: pushes recycle a small semaphore pool, so
    many small transfers serialize delivery. wo pushes are deferred past
    the batch-0 qT phase to keep the scalar engine free for qT copies.
  - y stores are one whole-tile DMA per row chunk, alternating queues.
"""

import os
import sys
from contextlib import ExitStack

import numpy as np

for _p in ("/opt/trn_rl_repo", "/opt/pypackages"):
    if _p not in sys.path:
        sys.path.append(_p)

import concourse.bacc as bacc
import concourse.mybir as mybir
import concourse.tile as tile
from concourse import bass_utils

B, N, C = 32, 1024, 768
H, D = 12, 64
P = 128
NCORES = 8
BS = B // NCORES          # batches per core
KC = C // P               # 6 contraction chunks over C
NM = N // P               # 8 chunks over N
F32 = mybir.dt.float32
BF16 = mybir.dt.bfloat16


def _emit(ctx, tc, xt_d, wqkv_d, wqt_d, wo_d, y_d):
    nc = tc.nc

    const = ctx.enter_context(tc.tile_pool(name="const", bufs=1))
    xt_pool = ctx.enter_context(tc.tile_pool(name="xtp", bufs=2 * KC))
    kv_pool = ctx.enter_context(tc.tile_pool(name="kvp", bufs=16))
    b_pool = ctx.enter_context(tc.tile_pool(name="bp", bufs=24))
    y_pool = ctx.enter_context(tc.tile_pool(name="yp", bufs=3))
    sm_pool = ctx.enter_context(tc.tile_pool(name="smp", bufs=6))
    psB = ctx.enter_context(tc.tile_pool(name="psB", bufs=3, space="PSUM"))
    # Dedicated PSUM ring for qT chains: they are woven into other phases
    # as filler, and sharing a ring with kv/B/y psums would stall them on
    # those phases' unfinished PSUM->SBUF copies (ring-wrap head-of-line).
    psQ = ctx.enter_context(tc.tile_pool(name="psQ", bufs=2, space="PSUM"))
    psZ = ctx.enter_context(tc.tile_pool(name="psZ", bufs=3, space="PSUM"))

    # Persistent block-diag lhsT tiles for the B matmul, zeroed once. Only
    # the diagonal blocks are rewritten per pair, so off-diag zeros persist.
    zeros = const.tile([P, P], F32, tag="zeros", name="zeros")
    nc.vector.memset(zeros[:], 0.0)
    a2_tiles = []
    for i in range(2):
        a2t = const.tile([P, P], BF16, tag=f"a2_{i}", name=f"a2_{i}")
        nc.vector.tensor_copy(a2t[:], zeros[:])
        a2_tiles.append(a2t)

    def load_xt(b):
        xT = [xt_pool.tile([P, N], BF16, tag="xT", name=f"xT{b}_{p}")
              for p in range(KC)]
        for p in range(KC):
            nc.sync.dma_start(xT[p][:], xt_d[b, p * P:(p + 1) * P, :])
        return xT

    # Startup DMA is bandwidth-bound, so issue transfers in strict
    # need-order split across the two HWDGE queues: xt(b0) on sync || wq on
    # scalar (they gate the qT phase), then wkv split across both queues
    # (gates kv), then wo and the xt(b1) prefetch which are needed later.
    # xt0 / wkv tiles interleave across BOTH queues in p-order: the first
    # kv chain consumes all p chunks of both, so pairwise arrival beats
    # loading each tensor on its own queue back-to-back. wqT (W_q
    # transposed, for the G = W_q @ B matmul) and wo are needed only from
    # the attention phase on, so they load after.
    xt0 = [xt_pool.tile([P, N], BF16, tag="xT", name=f"xT0_{p}")
           for p in range(KC)]
    wkv = [const.tile([P, 2 * C], BF16, tag=f"wkv{p}", name=f"wkv{p}")
           for p in range(KC)]
    for p in range(KC):
        e_x = nc.sync if p % 2 == 0 else nc.scalar
        e_w = nc.scalar if p % 2 == 0 else nc.sync
        e_x.dma_start(xt0[p][:], xt_d[0, p * P:(p + 1) * P, :])
        e_w.dma_start(wkv[p][:], wqkv_d[p * P:(p + 1) * P, C:3 * C])
    wo = [const.tile([P, C], BF16, tag=f"wo{p}", name=f"wo{p}")
          for p in range(KC)]
    wqT = [const.tile([P, C], BF16, tag=f"wqT{p}", name=f"wqT{p}")
           for p in range(KC)]
    for p in range(KC):
        eng = nc.sync if p % 2 == 0 else nc.scalar
        eng.dma_start(wo[p][:], wo_d[p * P:(p + 1) * P, :])
        eng2 = nc.scalar if p % 2 == 0 else nc.sync
        eng2.dma_start(wqT[p][:], wqt_d[p * P:(p + 1) * P, :])

    xt_next = xt0

    def make_kv(b, xT):
        """Allocate kv tiles now; return (tiles, list of per-chain thunks)
        so the chains can be woven into another phase's PE gaps."""
        kv = [kv_pool.tile([P, 2 * C], BF16, tag="kv", name=f"kv{b}_{m}")
              for m in range(NM)]

        def chain(m, f):
            def th():
                ps = psQ.tile([P, 512], F32, tag="psQ",
                              name=f"pskv{b}_{m}_{f}", space="PSUM")
                for p in range(KC):
                    nc.tensor.matmul(
                        ps[:],
                        xT[p][:, m * P:(m + 1) * P],
                        wkv[p][:, f * 512:(f + 1) * 512],
                        start=(p == 0), stop=(p == KC - 1))
                if f == 2:
                    nc.scalar.copy(kv[m][:, f * 512:(f + 1) * 512], ps[:])
                else:
                    nc.vector.tensor_copy(kv[m][:, f * 512:(f + 1) * 512],
                                          ps[:])
            return th

        return kv, [chain(m, f) for m in range(NM) for f in range(3)]

    def emit_kv0_split(kv, mk_chain_args, xT):
        """Batch-0 startup: the first NSPLIT chains are emitted as split
        halves (p=0..2 first, p=3..5 + copy later) across the psB+psQ
        rings, so the PE has runnable work as soon as the early (xt_p,
        wkv_p) DMA pairs land instead of waiting for the last pair."""
        NSPLIT = 5
        halves = []
        for i, (m, f) in enumerate(mk_chain_args[:NSPLIT]):
            pool = psB if i < 3 else psQ
            ps = pool.tile([P, 512], F32, tag="psB" if i < 3 else "psQ",
                           name=f"pskv0_{m}_{f}", space="PSUM")
            for p in range(KC // 2):
                nc.tensor.matmul(
                    ps[:],
                    xT[p][:, m * P:(m + 1) * P],
                    wkv[p][:, f * 512:(f + 1) * 512],
                    start=(p == 0), stop=False)
            halves.append((m, f, ps))
        for m, f, ps in halves:
            for p in range(KC // 2, KC):
                nc.tensor.matmul(
                    ps[:],
                    xT[p][:, m * P:(m + 1) * P],
                    wkv[p][:, f * 512:(f + 1) * 512],
                    start=False, stop=(p == KC - 1))
            if f == 2:
                nc.scalar.copy(kv[m][:, f * 512:(f + 1) * 512], ps[:])
            else:
                nc.vector.tensor_copy(kv[m][:, f * 512:(f + 1) * 512], ps[:])

    def emit_attention(b, kv, filler=()):
        """filler: thunks of independent PE work (next batch's qT chains)
        woven into the pipeline tail, where no z chains remain to keep the
        PE busy while the ACT/DVE softmax of the last pairs drains."""
        fill_iter = iter(filler)
        Bt = []
        LOOKAHEAD = 2
        zps_pair = {}
        for step in range(KC + LOOKAHEAD):
            if step >= KC - 2:
                for _ in range(3 if step < KC else 4):
                    th = next(fill_iter, None)
                    if th is not None:
                        th()
            if step < KC:
                pr = step
                # z for both heads of the pair in one chain: lhsT = the
                # pair's k (M=128), rhs = the pair's v (free=128). Head 2pr
                # lands on psum rows/cols 0:64, head 2pr+1 on 64:128; the
                # off-diag blocks are cross-head garbage that stays unused.
                zps = psZ.tile([P, P], F32, tag="z", name=f"z{b}_{pr}",
                               space="PSUM")
                zps_pair[pr] = zps
                for m in range(NM):
                    nc.tensor.matmul(
                        zps[:],
                        kv[m][:, 2 * pr * D:(2 * pr + 2) * D],
                        kv[m][:, C + 2 * pr * D:C + (2 * pr + 2) * D],
                        start=(m == 0), stop=(m == NM - 1))
            if step < LOOKAHEAD:
                continue
            pr = step - LOOKAHEAD
            a2 = a2_tiles[pr % 2]
            zps = zps_pair.pop(pr)
            # One exp over the whole tile (garbage off-diag included: values
            # are ~exp(+-16), finite in fp32, and never read afterwards).
            aex = sm_pool.tile([P, P], F32, tag="aex", name=f"aex{b}_{pr}")
            nc.scalar.activation(aex[:], zps[:],
                                 mybir.ActivationFunctionType.Exp,
                                 bias=0.0, scale=0.125)
            ssum = sm_pool.tile([P, 1], F32, tag="ssum", name=f"ss{b}_{pr}")
            for j in range(2):
                rb = j * D
                nc.vector.tensor_reduce(ssum[rb:rb + D, :],
                                        aex[rb:rb + D, rb:rb + D],
                                        mybir.AxisListType.X,
                                        mybir.AluOpType.add)
            rinv = sm_pool.tile([P, 1], F32, tag="rinv", name=f"ri{b}_{pr}")
            nc.vector.reciprocal(rinv[:], ssum[:])
            # a2 = A^T for the pair (block-diag, bf16): the softmax 1/sum is
            # applied by the per-partition scale of this copy.
            for j in range(2):
                rb = j * D
                nc.vector.tensor_scalar_mul(a2[rb:rb + D, rb:rb + D],
                                            aex[rb:rb + D, rb:rb + D],
                                            rinv[rb:rb + D, :])
            # B_pr = a2^T @ w_out rows of this pair: contraction depth is
            # only 128 (the pair's d-rows), free = C split in two.
            bt = b_pool.tile([P, C], BF16, tag="B", name=f"B{b}_{pr}")
            Bt.append(bt)
            for f in range(2):
                ps = psB.tile([P, 384], F32, tag="psB", name=f"psb{b}_{pr}_{f}",
                              space="PSUM")
                nc.tensor.matmul(ps[:], a2[:],
                                 wo[pr][:, f * 384:(f + 1) * 384],
                                 start=True, stop=True)
                if f == 0:
                    nc.scalar.copy(bt[:, f * 384:(f + 1) * 384], ps[:])
                else:
                    nc.vector.tensor_copy(bt[:, f * 384:(f + 1) * 384], ps[:])
        for th in fill_iter:
            th()
        return Bt

    def emit_g(b, Bt):
        G = []
        for po in range(KC):
            gt = b_pool.tile([P, C], BF16, tag="G", name=f"G{b}_{po}")
            G.append(gt)
            for f in range(2):
                ps = psB.tile([P, 384], F32, tag="psB", name=f"psg{b}_{po}_{f}",
                              space="PSUM")
                for p in range(KC):
                    nc.tensor.matmul(
                        ps[:],
                        wqT[p][:, po * P:(po + 1) * P],
                        Bt[p][:, f * 384:(f + 1) * 384],
                        start=(p == 0), stop=(p == KC - 1))
                if f == 0:
                    nc.vector.tensor_copy(gt[:, f * 384:(f + 1) * 384], ps[:])
                else:
                    nc.scalar.copy(gt[:, f * 384:(f + 1) * 384], ps[:])
        return G

    def y_chain(b, xT, G, m):
        def th():
            yt = y_pool.tile([P, C], BF16, tag="y", name=f"y{b}_{m}")
            for f in range(2):
                ps = psB.tile([P, 384], F32, tag="psB", name=f"psy{b}_{m}_{f}",
                              space="PSUM")
                for p in range(KC):
                    nc.tensor.matmul(
                        ps[:],
                        xT[p][:, m * P:(m + 1) * P],
                        G[p][:, f * 384:(f + 1) * 384],
                        start=(p == 0), stop=(p == KC - 1))
                if f == 0:
                    nc.vector.tensor_copy(yt[:, f * 384:(f + 1) * 384], ps[:])
                else:
                    nc.scalar.copy(yt[:, f * 384:(f + 1) * 384], ps[:])
            # One whole-tile store per row chunk (descriptor pushes are
            # expensive), alternating between the two HWDGE queues so the
            # final batch's writeback drains at full aggregate bandwidth.
            # The last batch stores halves as they finish: nothing overlaps
            # the final drain, so starting it earlier shortens the tail.
            eng = nc.sync if m % 2 == 0 else nc.scalar
            if b == BS - 1:
                for f in range(2):
                    eng.dma_start(y_d[b, m * P:(m + 1) * P,
                                      f * 384:(f + 1) * 384],
                                  yt[:, f * 384:(f + 1) * 384])
            else:
                eng.dma_start(y_d[b, m * P:(m + 1) * P, :], yt[:])
        return th

    def emit_y(b, xT, G, hold=0):
        ths = [y_chain(b, xT, G, m) for m in range(NM)]
        for th in ths[:NM - hold]:
            th()
        return ths[NM - hold:]

    # Batch pipeline, software-pipelined by one batch: the NEXT batch's kv
    # chains are woven into the attention tail of the current batch, where
    # the PE would otherwise idle waiting on the ACT/DVE softmax drain; the
    # rest are emitted just before the next attention phase. The last batch
    # weaves the previous batch's held-back y chains instead.
    kv, kv_thunks = make_kv(0, xt0)
    kv0_args = [(m, f) for m in range(NM) for f in range(3)]
    emit_kv0_split(kv, kv0_args, xt0)
    for th in kv_thunks[5:]:
        th()
    xT = xt0
    held_y = ()
    for b in range(BS):
        # Prefetch next batch's xT now: its pool slots free up as the kv
        # chains above retire, and these loads sit AHEAD of this batch's y
        # stores on the sync queue so they can't be head-of-line blocked.
        if b + 1 < BS:
            xt_next = load_xt(b + 1)
            kv_next, kv_thunks = make_kv(b + 1, xt_next)
        else:
            kv_next, kv_thunks = None, ()
        n_weave = min(6, len(kv_thunks))
        Bt = emit_attention(b, kv,
                            filler=list(kv_thunks[:n_weave]) + list(held_y))
        G = emit_g(b, Bt)
        held_y = emit_y(b, xT, G, hold=6 if b == BS - 2 else 0)
        for th in kv_thunks[n_weave:]:
            th()
        kv, xT = kv_next, xt_next


_BUILD_CACHE = {}


def build_program():
    if "nc" in _BUILD_CACHE:
        return _BUILD_CACHE["nc"]
    nc = bacc.Bacc("TRN2", target_bir_lowering=False, debug=False,
                   num_devices=NCORES)
    xt_d = nc.dram_tensor("xt", [BS, C, N], BF16, kind="ExternalInput").ap()
    wqkv_d = nc.dram_tensor("w_qkv", [C, 3 * C], BF16, kind="ExternalInput").ap()
    wqt_d = nc.dram_tensor("w_qt", [C, C], BF16, kind="ExternalInput").ap()
    wo_d = nc.dram_tensor("w_out", [C, C], BF16, kind="ExternalInput").ap()
    y_d = nc.dram_tensor("y", [BS, N, C], BF16, kind="ExternalOutput").ap()
    with tile.TileContext(nc) as tc:
        with ExitStack() as ctx:
            _emit(ctx, tc, xt_d, wqkv_d, wqt_d, wo_d, y_d)
    nc.compile()
    _BUILD_CACHE["nc"] = nc
    return nc


def make_in_maps(x, w_qkv, w_out):
    import ml_dtypes
    bf16 = ml_dtypes.bfloat16
    x = np.asarray(x, dtype=np.float32)
    w_qkv = np.asarray(w_qkv, dtype=np.float32)
    w_qt = np.ascontiguousarray(w_qkv[:, :C].T).astype(bf16)
    w_qkv = np.ascontiguousarray(w_qkv).astype(bf16)
    w_out = np.ascontiguousarray(np.asarray(w_out, dtype=np.float32)).astype(bf16)
    return [
        {"xt": np.ascontiguousarray(
            x[i * BS:(i + 1) * BS].transpose(0, 2, 1)).astype(bf16),
         "w_qkv": w_qkv, "w_qt": w_qt, "w_out": w_out}
        for i in range(NCORES)
    ]


def kernel(x, w_qkv, b_qkv=None, w_out=None, b_out=None, **_unused):
    nc = build_program()
    in_maps = make_in_maps(x, w_qkv, w_out)
    res = bass_utils.run_bass_kernel_spmd(nc, in_maps,
                                          core_ids=list(range(NCORES)))
    y = np.concatenate([res.results[i]["y"] for i in range(NCORES)], axis=0)
    return np.asarray(y, dtype=np.float32)


# revision 28
# speedup vs baseline: 1.1489x; 1.0140x over previous
"""ChannelMHSA on Trainium2 (Bass/Tile), data-parallel over batch on 8 cores.

Reference computation (per batch b of x [N, C]):
    qkv  = x @ w_qkv                      # [N, 3C], columns ordered (s, h, d)
    q, k, v per head h: [N, D]
    z_h  = k_h^T @ v_h / sqrt(D)          # [D, D]
    A_h  = softmax(z_h, axis=-1)
    T_h  = A_h @ q_h^T                    # [D, N]
    out[n, h*D+d] = T_h[d, n]
    y    = out @ w_out                    # [N, C]

b_qkv / b_out are all-zero by construction (see input spec) and are ignored.

Kernel layout choices per core (BS=4 batches):
  - Everything runs in bf16 on the PE (1 cycle/row at any free size) with
    fp32 PSUM accumulation; measured end-to-end error vs the fp32
    reference ~8e-3 (tolerance 2e-2). Host pre-casts x / w_qkv / w_out to
    bf16, which also halves the startup DMA bytes.
  - x is transposed on the HOST: the kernel uploads xT [C, N] per batch
    directly, so the PE never runs transpose matmuls and there is no
    xin-DMA stall at batch boundaries. xt_pool holds two batches so batch
    b+1's xT prefetches during batch b's attention/output phases.
  - qT = w_q^T @ x^T computed C-major directly (lhsT = w_q chunks,
    rhs = xT chunks), so q never needs a separate transpose.
  - kv = x @ w_qkv[:, C:3C] computed N-major (lhsT = xT chunks).
  - The output projection is FUSED through the attention:
        y = out @ w_out = q~ @ B,  B_h = A_h^T @ w_out[hD:(h+1)D, :]
    B costs only D-deep contractions (4.6k PE cycles/batch vs 12.3k for
    the A @ q^T route) and y's lhsT becomes qT itself - the T tensor and
    its PSUM->SBUF copies disappear entirely.
  - z per head pair is one chain (lhsT = the pair's k, rhs = the pair's v,
    free=128), emitted LOOKAHEAD=2 pairs ahead of the softmax so the PE
    never waits on ACT. softmax: one exp over the whole [128,128] zps
    (off-diagonal garbage is harmless and ignored), per-block row-sums on
    DVE (free-axis tensor_reduce), and the 1/sum folds into the DVE copy
    that writes the block-diag bf16 a2 = A^T tile feeding the B matmul.
    No max-shift needed: |z/8| is small enough for fp32 exp.
  - Startup DMA is need-ordered across the two HWDGE queues (sync/scalar)
    as FEW, BIG descriptors: pushes recycle a small semaphore pool, so
    many small transfers serialize delivery. wo pushes are deferred past
    the batch-0 qT phase to keep the scalar engine free for qT copies.
  - y stores are one whole-tile DMA per row chunk, alternating queues.
"""

import os
import sys
from contextlib import ExitStack

import numpy as np

for _p in ("/opt/trn_rl_repo", "/opt/pypackages"):
    if _p not in sys.path:
        sys.path.append(_p)

import concourse.bacc as bacc
import concourse.mybir as mybir
import concourse.tile as tile
from concourse import bass_utils

B, N, C = 32, 1024, 768
H, D = 12, 64
P = 128
NCORES = 8
BS = B // NCORES          # batches per core
KC = C // P               # 6 contraction chunks over C
NM = N // P               # 8 chunks over N
F32 = mybir.dt.float32
BF16 = mybir.dt.bfloat16


def _emit(ctx, tc, xt_d, wqkv_d, wqt_d, wo_d, y_d):
    nc = tc.nc

    const = ctx.enter_context(tc.tile_pool(name="const", bufs=1))
    xt_pool = ctx.enter_context(tc.tile_pool(name="xtp", bufs=2 * KC))
    kv_pool = ctx.enter_context(tc.tile_pool(name="kvp", bufs=16))
    b_pool = ctx.enter_context(tc.tile_pool(name="bp", bufs=24))
    y_pool = ctx.enter_context(tc.tile_pool(name="yp", bufs=3))
    sm_pool = ctx.enter_context(tc.tile_pool(name="smp", bufs=6))
    psB = ctx.enter_context(tc.tile_pool(name="psB", bufs=3, space="PSUM"))
    # Dedicated PSUM ring for qT chains: they are woven into other phases
    # as filler, and sharing a ring with kv/B/y psums would stall them on
    # those phases' unfinished PSUM->SBUF copies (ring-wrap head-of-line).
    psQ = ctx.enter_context(tc.tile_pool(name="psQ", bufs=2, space="PSUM"))
    psZ = ctx.enter_context(tc.tile_pool(name="psZ", bufs=3, space="PSUM"))

    # Persistent block-diag lhsT tiles for the B matmul, zeroed once. Only
    # the diagonal blocks are rewritten per pair, so off-diag zeros persist.
    zeros = const.tile([P, P], F32, tag="zeros", name="zeros")
    nc.vector.memset(zeros[:], 0.0)
    a2_tiles = []
    for i in range(2):
        a2t = const.tile([P, P], BF16, tag=f"a2_{i}", name=f"a2_{i}")
        nc.vector.tensor_copy(a2t[:], zeros[:])
        a2_tiles.append(a2t)

    def load_xt(b):
        xT = [xt_pool.tile([P, N], BF16, tag="xT", name=f"xT{b}_{p}")
              for p in range(KC)]
        for p in range(KC):
            nc.sync.dma_start(xT[p][:], xt_d[b, p * P:(p + 1) * P, :])
        return xT

    # Startup DMA is bandwidth-bound, so issue transfers in strict
    # need-order split across the two HWDGE queues: xt(b0) on sync || wq on
    # scalar (they gate the qT phase), then wkv split across both queues
    # (gates kv), then wo and the xt(b1) prefetch which are needed later.
    # xt0 / wkv tiles interleave across BOTH queues in p-order: the first
    # kv chain consumes all p chunks of both, so pairwise arrival beats
    # loading each tensor on its own queue back-to-back. wqT (W_q
    # transposed, for the G = W_q @ B matmul) and wo are needed only from
    # the attention phase on, so they load after.
    xt0 = [xt_pool.tile([P, N], BF16, tag="xT", name=f"xT0_{p}")
           for p in range(KC)]
    wkv = [const.tile([P, 2 * C], BF16, tag=f"wkv{p}", name=f"wkv{p}")
           for p in range(KC)]
    for p in range(KC):
        e_x = nc.sync if p % 2 == 0 else nc.scalar
        e_w = nc.scalar if p % 2 == 0 else nc.sync
        e_x.dma_start(xt0[p][:], xt_d[0, p * P:(p + 1) * P, :])
        e_w.dma_start(wkv[p][:], wqkv_d[p * P:(p + 1) * P, C:3 * C])
    wo = [const.tile([P, C], BF16, tag=f"wo{p}", name=f"wo{p}")
          for p in range(KC)]
    wqT = [const.tile([P, C], BF16, tag=f"wqT{p}", name=f"wqT{p}")
           for p in range(KC)]
    for p in range(KC):
        eng = nc.sync if p % 2 == 0 else nc.scalar
        eng.dma_start(wo[p][:], wo_d[p * P:(p + 1) * P, :])
        eng2 = nc.scalar if p % 2 == 0 else nc.sync
        eng2.dma_start(wqT[p][:], wqt_d[p * P:(p + 1) * P, :])

    xt_next = xt0

    def make_kv(b, xT):
        """Allocate kv tiles now; return (tiles, list of per-chain thunks)
        so the chains can be woven into another phase's PE gaps."""
        kv = [kv_pool.tile([P, 2 * C], BF16, tag="kv", name=f"kv{b}_{m}")
              for m in range(NM)]

        def chain(m, f):
            def th():
                ps = psQ.tile([P, 512], F32, tag="psQ",
                              name=f"pskv{b}_{m}_{f}", space="PSUM")
                for p in range(KC):
                    nc.tensor.matmul(
                        ps[:],
                        xT[p][:, m * P:(m + 1) * P],
                        wkv[p][:, f * 512:(f + 1) * 512],
                        start=(p == 0), stop=(p == KC - 1))
                if f == 2:
                    nc.scalar.copy(kv[m][:, f * 512:(f + 1) * 512], ps[:])
                else:
                    nc.vector.tensor_copy(kv[m][:, f * 512:(f + 1) * 512],
                                          ps[:])
            return th

        return kv, [chain(m, f) for m in range(NM) for f in range(3)]

    def emit_kv0_split(kv, mk_chain_args, xT):
        """Batch-0 startup: the first NSPLIT chains are emitted as split
        halves (p=0..2 first, p=3..5 + copy later) across the psB+psQ
        rings, so the PE has runnable work as soon as the early (xt_p,
        wkv_p) DMA pairs land instead of waiting for the last pair."""
        NSPLIT = 8
        halves = []
        for i, (m, f) in enumerate(mk_chain_args[:NSPLIT]):
            # psZ's banks idle until the attention phase, so the startup
            # borrows them for three more open accumulation groups.
            pool, tag = ((psB, "psB") if i < 3 else
                         (psQ, "psQ") if i < 5 else (psZ, "z"))
            ps = pool.tile([P, 512], F32, tag=tag,
                           name=f"pskv0_{m}_{f}", space="PSUM")
            for p in range(KC // 2):
                nc.tensor.matmul(
                    ps[:],
                    xT[p][:, m * P:(m + 1) * P],
                    wkv[p][:, f * 512:(f + 1) * 512],
                    start=(p == 0), stop=False)
            halves.append((m, f, ps))
        for m, f, ps in halves:
            for p in range(KC // 2, KC):
                nc.tensor.matmul(
                    ps[:],
                    xT[p][:, m * P:(m + 1) * P],
                    wkv[p][:, f * 512:(f + 1) * 512],
                    start=False, stop=(p == KC - 1))
            if f == 2:
                nc.scalar.copy(kv[m][:, f * 512:(f + 1) * 512], ps[:])
            else:
                nc.vector.tensor_copy(kv[m][:, f * 512:(f + 1) * 512], ps[:])

    def emit_attention(b, kv, filler=()):
        """filler: thunks of independent PE work (next batch's qT chains)
        woven into the pipeline tail, where no z chains remain to keep the
        PE busy while the ACT/DVE softmax of the last pairs drains."""
        fill_iter = iter(filler)
        Bt = []
        LOOKAHEAD = 2
        zps_pair = {}
        for step in range(KC + LOOKAHEAD):
            if step >= KC - 2:
                for _ in range(3 if step < KC else 4):
                    th = next(fill_iter, None)
                    if th is not None:
                        th()
            if step < KC:
                pr = step
                # z for both heads of the pair in one chain: lhsT = the
                # pair's k (M=128), rhs = the pair's v (free=128). Head 2pr
                # lands on psum rows/cols 0:64, head 2pr+1 on 64:128; the
                # off-diag blocks are cross-head garbage that stays unused.
                zps = psZ.tile([P, P], F32, tag="z", name=f"z{b}_{pr}",
                               space="PSUM")
                zps_pair[pr] = zps
                for m in range(NM):
                    nc.tensor.matmul(
                        zps[:],
                        kv[m][:, 2 * pr * D:(2 * pr + 2) * D],
                        kv[m][:, C + 2 * pr * D:C + (2 * pr + 2) * D],
                        start=(m == 0), stop=(m == NM - 1))
            if step < LOOKAHEAD:
                continue
            pr = step - LOOKAHEAD
            a2 = a2_tiles[pr % 2]
            zps = zps_pair.pop(pr)
            # One exp over the whole tile (garbage off-diag included: values
            # are ~exp(+-16), finite in fp32, and never read afterwards).
            aex = sm_pool.tile([P, P], F32, tag="aex", name=f"aex{b}_{pr}")
            nc.scalar.activation(aex[:], zps[:],
                                 mybir.ActivationFunctionType.Exp,
                                 bias=0.0, scale=0.125)
            ssum = sm_pool.tile([P, 1], F32, tag="ssum", name=f"ss{b}_{pr}")
            for j in range(2):
                rb = j * D
                nc.vector.tensor_reduce(ssum[rb:rb + D, :],
                                        aex[rb:rb + D, rb:rb + D],
                                        mybir.AxisListType.X,
                                        mybir.AluOpType.add)
            rinv = sm_pool.tile([P, 1], F32, tag="rinv", name=f"ri{b}_{pr}")
            nc.vector.reciprocal(rinv[:], ssum[:])
            # a2 = A^T for the pair (block-diag, bf16): the softmax 1/sum is
            # applied by the per-partition scale of this copy.
            for j in range(2):
                rb = j * D
                nc.vector.tensor_scalar_mul(a2[rb:rb + D, rb:rb + D],
                                            aex[rb:rb + D, rb:rb + D],
                                            rinv[rb:rb + D, :])
            # B_pr = a2^T @ w_out rows of this pair: contraction depth is
            # only 128 (the pair's d-rows), free = C split in two.
            bt = b_pool.tile([P, C], BF16, tag="B", name=f"B{b}_{pr}")
            Bt.append(bt)
            for f in range(2):
                ps = psB.tile([P, 384], F32, tag="psB", name=f"psb{b}_{pr}_{f}",
                              space="PSUM")
                nc.tensor.matmul(ps[:], a2[:],
                                 wo[pr][:, f * 384:(f + 1) * 384],
                                 start=True, stop=True)
                if f == 0:
                    nc.scalar.copy(bt[:, f * 384:(f + 1) * 384], ps[:])
                else:
                    nc.vector.tensor_copy(bt[:, f * 384:(f + 1) * 384], ps[:])
        for th in fill_iter:
            th()
        return Bt

    def emit_g(b, Bt):
        G = []
        for po in range(KC):
            gt = b_pool.tile([P, C], BF16, tag="G", name=f"G{b}_{po}")
            G.append(gt)
            for f in range(2):
                ps = psB.tile([P, 384], F32, tag="psB", name=f"psg{b}_{po}_{f}",
                              space="PSUM")
                for p in range(KC):
                    nc.tensor.matmul(
                        ps[:],
                        wqT[p][:, po * P:(po + 1) * P],
                        Bt[p][:, f * 384:(f + 1) * 384],
                        start=(p == 0), stop=(p == KC - 1))
                if f == 0:
                    nc.vector.tensor_copy(gt[:, f * 384:(f + 1) * 384], ps[:])
                else:
                    nc.scalar.copy(gt[:, f * 384:(f + 1) * 384], ps[:])
        return G

    def y_chain(b, xT, G, m):
        def th():
            yt = y_pool.tile([P, C], BF16, tag="y", name=f"y{b}_{m}")
            for f in range(2):
                ps = psB.tile([P, 384], F32, tag="psB", name=f"psy{b}_{m}_{f}",
                              space="PSUM")
                for p in range(KC):
                    nc.tensor.matmul(
                        ps[:],
                        xT[p][:, m * P:(m + 1) * P],
                        G[p][:, f * 384:(f + 1) * 384],
                        start=(p == 0), stop=(p == KC - 1))
                if f == 0:
                    nc.vector.tensor_copy(yt[:, f * 384:(f + 1) * 384], ps[:])
                else:
                    nc.scalar.copy(yt[:, f * 384:(f + 1) * 384], ps[:])
            # One whole-tile store per row chunk (descriptor pushes are
            # expensive), alternating between the two HWDGE queues so the
            # final batch's writeback drains at full aggregate bandwidth.
            # The last batch stores halves as they finish: nothing overlaps
            # the final drain, so starting it earlier shortens the tail.
            eng = nc.sync if m % 2 == 0 else nc.scalar
            if b == BS - 1:
                for f in range(2):
                    eng.dma_start(y_d[b, m * P:(m + 1) * P,
                                      f * 384:(f + 1) * 384],
                                  yt[:, f * 384:(f + 1) * 384])
            else:
                eng.dma_start(y_d[b, m * P:(m + 1) * P, :], yt[:])
        return th

    def emit_y(b, xT, G, hold=0):
        ths = [y_chain(b, xT, G, m) for m in range(NM)]
        for th in ths[:NM - hold]:
            th()
        return ths[NM - hold:]

    # Batch pipeline, software-pipelined by one batch: the NEXT batch's kv
    # chains are woven into the attention tail of the current batch, where
    # the PE would otherwise idle waiting on the ACT/DVE softmax drain; the
    # rest are emitted just before the next attention phase. The last batch
    # weaves the previous batch's held-back y chains instead.
    kv, kv_thunks = make_kv(0, xt0)
    kv0_args = [(m, f) for m in range(NM) for f in range(3)]
    emit_kv0_split(kv, kv0_args, xt0)
    for th in kv_thunks[8:]:
        th()
    xT = xt0
    held_y = ()
    for b in range(BS):
        # Prefetch next batch's xT now: its pool slots free up as the kv
        # chains above retire, and these loads sit AHEAD of this batch's y
        # stores on the sync queue so they can't be head-of-line blocked.
        if b + 1 < BS:
            xt_next = load_xt(b + 1)
            kv_next, kv_thunks = make_kv(b + 1, xt_next)
        else:
            kv_next, kv_thunks = None, ()
        n_weave = min(8, len(kv_thunks))
        Bt = emit_attention(b, kv,
                            filler=list(kv_thunks[:n_weave]) + list(held_y))
        G = emit_g(b, Bt)
        held_y = emit_y(b, xT, G, hold=6 if b == BS - 2 else 0)
        for th in kv_thunks[n_weave:]:
            th()
        kv, xT = kv_next, xt_next


_BUILD_CACHE = {}


def build_program():
    if "nc" in _BUILD_CACHE:
        return _BUILD_CACHE["nc"]
    nc = bacc.Bacc("TRN2", target_bir_lowering=False, debug=False,
                   num_devices=NCORES)
    xt_d = nc.dram_tensor("xt", [BS, C, N], BF16, kind="ExternalInput").ap()
    wqkv_d = nc.dram_tensor("w_qkv", [C, 3 * C], BF16, kind="ExternalInput").ap()
    wqt_d = nc.dram_tensor("w_qt", [C, C], BF16, kind="ExternalInput").ap()
    wo_d = nc.dram_tensor("w_out", [C, C], BF16, kind="ExternalInput").ap()
    y_d = nc.dram_tensor("y", [BS, N, C], BF16, kind="ExternalOutput").ap()
    with tile.TileContext(nc) as tc:
        with ExitStack() as ctx:
            _emit(ctx, tc, xt_d, wqkv_d, wqt_d, wo_d, y_d)
    nc.compile()
    _BUILD_CACHE["nc"] = nc
    return nc


def make_in_maps(x, w_qkv, w_out):
    import ml_dtypes
    bf16 = ml_dtypes.bfloat16
    x = np.asarray(x, dtype=np.float32)
    w_qkv = np.asarray(w_qkv, dtype=np.float32)
    w_qt = np.ascontiguousarray(w_qkv[:, :C].T).astype(bf16)
    w_qkv = np.ascontiguousarray(w_qkv).astype(bf16)
    w_out = np.ascontiguousarray(np.asarray(w_out, dtype=np.float32)).astype(bf16)
    return [
        {"xt": np.ascontiguousarray(
            x[i * BS:(i + 1) * BS].transpose(0, 2, 1)).astype(bf16),
         "w_qkv": w_qkv, "w_qt": w_qt, "w_out": w_out}
        for i in range(NCORES)
    ]


def kernel(x, w_qkv, b_qkv=None, w_out=None, b_out=None, **_unused):
    nc = build_program()
    in_maps = make_in_maps(x, w_qkv, w_out)
    res = bass_utils.run_bass_kernel_spmd(nc, in_maps,
                                          core_ids=list(range(NCORES)))
    y = np.concatenate([res.results[i]["y"] for i in range(NCORES)], axis=0)
    return np.asarray(y, dtype=np.float32)
